# revision 19
# baseline (speedup 1.0000x reference)
"""ChebyKAN layer on 8 Trainium2 NeuronCores.

y = einsum('dbi,dio->bo', cheby_basis(tanh(x)), cheby_coeffs)

Strategy (per core, data-parallel over batch):
  - each core takes 1024 rows of x (8192/8) and the full coeffs
  - x arrives pre-transposed ([i, b] layout) from the host; tanh on the
    scalar engine
  - cheby_coeffs arrive as bf16 (host cast): halves the dominant W DMA
    stream (the W tensor is streamed once per batch-half) and the bf16
    stationary operand gets fast weight load on the PE
  - Chebyshev basis built on-the-fly in fp32 on the vector engine and
    cast to bf16 (the BIR verifier requires both matmul operands to
    share a dtype; bf16 runs full rate, rel err ~2e-3 vs the 2e-2 gate)
  - degree 0 (T0 == 1) is folded via V-fold: V[i',o] = sum_k W0[i'+128k,o]
    is pre-reduced on the vector engine (7 adds, off the PE), and each
    half accumulates it with 8 mid-stream start=False matmuls
    V.T @ ones[128,512]; replaces 128 full matmuls (1/9 of PE work)
    with 16 and adds zero head-of-pipe serialization
  - contraction: stationary = W[d, i-tile, o-tile] bf16, moving =
    T_d[i-tile, b-half] bf16; psum holds y.T chunks
    [o-tile 128, b-half 512] x 8 o-tiles = 8 banks
  - two b-halves of 512; W streamed from HBM once per half (bf16)
  - the last degree of each half runs ot-major so psum banks close
    progressively: evacuation + output DMA overlap the matmul stream
  - output is y.T per core; host transposes and concatenates
"""

import numpy as np
import ml_dtypes

import concourse.bass as bass
import concourse.tile as tile
from concourse import bacc, mybir
from concourse import bass_utils
from concourse.alu_op_type import AluOpType

N_CORES = 8
B = 8192
IC = 1024
OC = 1024
DEG = 8  # polynomial degree; DEG+1 = 9 basis terms
BC = B // N_CORES  # 1024 batch rows per core
P = 128
NI = IC // P  # 8 i-tiles
NO = OC // P  # 8 o-tiles
BH = BC // 2  # 512, b-half
F32 = mybir.dt.float32
BF16 = mybir.dt.bfloat16

# W slab granularity (in i-tiles): first degree of a half uses small
# head slabs so its first matmul's W-DMA chain is short.
_D1_SLABS = [1, 1, 2, 2, 2]
_D_SLABS = [2, 2, 2, 2]


def _build(tanh_scale: float, tanh_bias: float):
    nc = bacc.Bacc("TRN2", target_bir_lowering=False, debug=False, num_devices=N_CORES)

    xT_d = nc.dram_tensor("xT", [IC, BC], F32, kind="ExternalInput").ap()
    w_d = nc.dram_tensor("w", [DEG + 1, IC, OC], BF16, kind="ExternalInput").ap()
    yt_d = nc.dram_tensor("yt", [OC, BC], F32, kind="ExternalOutput").ap()

    with tile.TileContext(nc) as tc:
        with (
            tc.tile_pool(name="const", bufs=1) as constp,
            tc.tile_pool(name="xin", bufs=5) as xinp,
            tc.tile_pool(name="xt", bufs=2) as xtp,
            tc.tile_pool(name="state", bufs=3) as statep,
            tc.tile_pool(name="prod", bufs=2) as prodp,
            tc.tile_pool(name="tr", bufs=3) as trp,
            tc.tile_pool(name="wstage", bufs=8) as wstagep,
            tc.tile_pool(name="evac", bufs=3) as evacp,
            tc.tile_pool(name="ps", bufs=8, space=bass.MemorySpace.PSUM) as psp,
        ):
            ones512 = constp.tile([P, BH], BF16, tag="ones")
            nc.vector.memset(ones512[:], 1.0)
            v_bf = constp.tile([P, OC], BF16, tag="vbf")  # V: d0 fold, bf16

            def emit_w_slabs(h, d, slab_sizes, it0=0, tag=None):
                """DMA W[d] i-tile slabs (bf16, used directly as stationary);
                returns [(first_it, ntiles, tile), ...]."""
                out = []
                for ws, nt in enumerate(slab_sizes):
                    wst = wstagep.tile(
                        [P, nt * OC],
                        BF16,
                        tag="wstage",
                        name=f"wst_{h}_{tag or d}_{ws}_{it0}",
                    )
                    nc.sync.dma_start(
                        wst[:].rearrange("p (il o) -> p il o", il=nt),
                        w_d[d, it0 * P : (it0 + nt) * P, :].rearrange(
                            "(il p) o -> p il o", p=P
                        ),
                    )
                    out.append((it0, nt, wst))
                    it0 += nt
                return out

            # ---- x.T chunk DMAs (batched up-front on a chosen queue) ----
            def emit_x_dmas(h, chunks, it0=0, eng=None):
                eng = eng or nc.sync
                tiles = []
                for nt in chunks:
                    xst = xinp.tile(
                        [P, nt * BH], F32, tag="xin", name=f"xs_{h}_{it0}"
                    )
                    eng.dma_start(
                        xst[:].rearrange("p (il b) -> p il b", il=nt),
                        xT_d[
                            it0 * P : (it0 + nt) * P, h * BH : (h + 1) * BH
                        ].rearrange("(il p) b -> p il b", p=P),
                    )
                    tiles.append((it0, nt, xst))
                    it0 += nt
                return tiles

            # ---- tanh (scalar) + T1 bf16 cast per staged chunk ----
            def emit_tanh_chunks(h, xt, tr1, xtiles, tr1_eng="vector"):
                for it0, nt, xst in xtiles:
                    sl = slice(it0 * BH, (it0 + nt) * BH)
                    nc.scalar.activation(
                        xt[:, sl],
                        xst[:],
                        mybir.ActivationFunctionType.Tanh,
                        bias=tanh_bias,
                        scale=tanh_scale,
                    )
                    if tr1_eng == "vector":
                        nc.vector.tensor_copy(tr1[:, sl], xt[:, sl])
                    else:
                        nc.scalar.activation(
                            tr1[:, sl], xt[:, sl], mybir.ActivationFunctionType.Copy
                        )

            def alloc_xt(h):
                xt = xtp.tile([P, NI * BH], F32, tag="xt", name=f"xt_{h}")
                tr1 = trp.tile([P, NI * BH], BF16, tag="tr1", name=f"tr_{h}_1", bufs=2)
                return xt, tr1

            def emit_matmuls(accs, wr_slabs, d, tr_d):
                if d == DEG:
                    # whole last degree ot-major: each acc's accumulation
                    # closes early so psum evac + y DMA overlap the stream
                    for ot in range(NO):
                        for si, (it0, nt, wst) in enumerate(wr_slabs):
                            for il in range(nt):
                                it = it0 + il
                                nc.tensor.matmul(
                                    accs[ot][:],
                                    wst[:, il * OC + ot * P : il * OC + (ot + 1) * P],
                                    tr_d[:, it * BH : (it + 1) * BH],
                                    start=False,
                                    stop=(it == NI - 1),
                                )
                    return
                for si, (it0, nt, wst) in enumerate(wr_slabs):
                    for il in range(nt):
                        it = it0 + il
                        rhs = tr_d[:, it * BH : (it + 1) * BH]
                        for ot in range(NO):
                            nc.tensor.matmul(
                                accs[ot][:],
                                wst[:, il * OC + ot * P : il * OC + (ot + 1) * P],
                                rhs,
                                start=(d == 1 and it == 0),
                                stop=False,
                            )

            def emit_d0(accs):
                # d0 fold: acc[ot] += V.T @ ones  (adds y0[o] to every b col)
                for ot in range(NO):
                    nc.tensor.matmul(
                        accs[ot][:],
                        v_bf[:, ot * P : (ot + 1) * P],
                        ones512[:],
                        start=False,
                        stop=False,
                    )

            # ---- head: x chunks on the scalar DMA queue (parallel to the
            # W stream on sync); all dma_starts queued before the tanh chain
            # so the scalar engine FIFO never blocks a transfer start ----
            xt0, tr1_0 = alloc_xt(0)
            x0_tiles = emit_x_dmas(0, chunks=[1, 1, 2, 2, 2], eng=nc.scalar)
            d1_head = emit_w_slabs(0, 1, _D1_SLABS[:2])
            emit_tanh_chunks(0, xt0, tr1_0, x0_tiles)

            xts = [(xt0, tr1_0), None]

            for h in range(2):
                if h == 0:
                    d1_slabs_h0 = d1_head + emit_w_slabs(
                        0, 1, _D1_SLABS[2:], it0=2, tag="1b"
                    )
                xt, tr1 = xts[h]
                # ---- accumulation psum tiles: y.T chunk per o-tile ----
                accs = [
                    psp.tile([P, BH], F32, tag="ps", name=f"acc_h{h}_o{ot}")
                    for ot in range(NO)
                ]

                # ---- degree loop (d0 folded mid-stream via emit_d0) ----
                t_m1 = xt  # T_{d-1} (fp32 slab)
                t_m2 = None  # T_{d-2}
                QS = NI * BH // 4
                for d in range(1, DEG + 1):
                    if d == 1:
                        tr_d = tr1
                    else:
                        tr_d = trp.tile(
                            [P, NI * BH], BF16, tag="tr", name=f"tr_{h}_{d}"
                        )
                        t_new = statep.tile(
                            [P, NI * BH], F32, tag="state", name=f"st_{h}_{d}"
                        )
                        for q in range(4):
                            sl = slice(q * QS, (q + 1) * QS)
                            prod = prodp.tile(
                                [P, QS], F32, tag="prod", name=f"prod_{h}_{d}_{q}"
                            )
                            nc.vector.scalar_tensor_tensor(
                                prod[:],
                                t_m1[:, sl],
                                2.0,
                                xt[:, sl],
                                AluOpType.mult,
                                AluOpType.mult,
                            )
                            if d == 2:
                                # T2 = 2*xt^2 - 1
                                nc.vector.tensor_scalar_sub(t_new[:, sl], prod[:], 1.0)
                            else:
                                nc.vector.tensor_sub(t_new[:, sl], prod[:], t_m2[:, sl])
                            if h == 1 and d == 2 and q < 2:
                                # boundary: scalar queue is draining h0's
                                # evacs; cast on DVE so d2's matmuls start
                                nc.vector.tensor_copy(tr_d[:, sl], t_new[:, sl])
                            else:
                                nc.scalar.activation(
                                    tr_d[:, sl],
                                    t_new[:, sl],
                                    mybir.ActivationFunctionType.Copy,
                                )
                        t_m2, t_m1 = t_m1, t_new

                    # ---- W stream + matmuls for this degree ----
                    if h == 0 and d == 1:
                        wr_slabs = d1_slabs_h0
                    else:
                        wr_slabs = emit_w_slabs(h, d, _D1_SLABS if d == 1 else _D_SLABS)
                    emit_matmuls(accs, wr_slabs, d, tr_d)

                    if h == 0 and d == 2:
                        xt1, tr1_1 = alloc_xt(1)
                        x1_tiles = emit_x_dmas(1, chunks=[2, 2, 2, 2], eng=nc.sync)
                        emit_tanh_chunks(1, xt1, tr1_1, x1_tiles, tr1_eng="scalar")
                        xts[1] = (xt1, tr1_1)
                    if h == 0 and d == 3:
                        # W0 stream for the V-fold, after d3's slabs (the DMA
                        # queue has plenty of slack mid-stream)
                        w0_slabs = emit_w_slabs(0, 0, _D_SLABS, tag="w0")
                    if h == 0 and d == 4:
                        # V = sum of W0's 8 i-tiles (DVE), then cast to bf16
                        vtmp = [
                            prodp.tile([P, OC], F32, tag="vtmp", name=f"v_{j}", bufs=2)
                            for j in range(2)
                        ]
                        w0_chunks = [
                            wst[:, il * OC : (il + 1) * OC]
                            for it0, nt, wst in w0_slabs
                            for il in range(nt)
                        ]
                        nc.vector.tensor_add(vtmp[0][:], w0_chunks[0], w0_chunks[1])
                        sel = 0
                        for k, ch in enumerate(w0_chunks[2:]):
                            sel = (k + 1) % 2
                            nc.vector.tensor_add(vtmp[sel][:], vtmp[k % 2][:], ch)
                        nc.vector.tensor_copy(v_bf[:], vtmp[sel][:])
                    if (h == 0 and d == 5) or (h == 1 and d == 2):
                        emit_d0(accs)

                # ---- evacuate psum -> SBUF -> y.T ----
                # h0: y DMAs all on the scalar queue (sync must keep feeding
                # h1's W stream). h1 (tail, W stream done): split across both.
                for ot in range(NO):
                    ev = evacp.tile([P, BH], F32, tag="evac", name=f"ev_h{h}_o{ot}")
                    if ot % 2 == 0:
                        nc.vector.tensor_copy(ev[:], accs[ot][:])
                    else:
                        nc.scalar.activation(
                            ev[:], accs[ot][:], mybir.ActivationFunctionType.Copy
                        )
                    dma_eng = nc.sync if (h == 1 and ot % 2 == 0) else nc.scalar
                    dma_eng.dma_start(
                        yt_d[ot * P : (ot + 1) * P, h * BH : (h + 1) * BH],
                        ev[:],
                    )

    nc.compile()
    return nc


_CACHE: dict = {}


def make_in_maps(x, w):
    w_bf = np.ascontiguousarray(np.asarray(w, dtype=np.float32)).astype(
        ml_dtypes.bfloat16
    )
    return [
        {"xT": np.ascontiguousarray(x[c * BC : (c + 1) * BC].T), "w": w_bf}
        for c in range(N_CORES)
    ]


def kernel(x, cheby_coeffs, tanh_scale, tanh_bias):
    x = np.ascontiguousarray(np.asarray(x, dtype=np.float32))
    ts = float(np.asarray(tanh_scale))
    tb = float(np.asarray(tanh_bias))

    key = (ts, tb)
    if key not in _CACHE:
        _CACHE[key] = _build(ts, tb)
    nc = _CACHE[key]

    in_maps = make_in_maps(x, cheby_coeffs)
    res = bass_utils.run_bass_kernel_spmd(
        nc, in_maps, core_ids=list(range(N_CORES)), trace=False
    )

    y = np.empty((B, OC), dtype=np.float32)
    for c in range(N_CORES):
        y[c * BC : (c + 1) * BC, :] = res.results[c]["yt"].T
    return y


# revision 23
# speedup vs baseline: 1.0335x; 1.0335x over previous
"""ChebyKAN layer on 8 Trainium2 NeuronCores.

y = einsum('dbi,dio->bo', cheby_basis(tanh(x)), cheby_coeffs)

Strategy (per core, data-parallel over batch):
  - each core takes 1024 rows of x (8192/8) and the full coeffs
  - x arrives pre-transposed ([i, b] layout) from the host; tanh on the
    scalar engine
  - cheby_coeffs arrive as bf16 (host cast): halves the dominant W DMA
    stream (the W tensor is streamed once per batch-half) and the bf16
    stationary operand gets fast weight load on the PE
  - Chebyshev basis built on-the-fly in fp32 on the vector engine and
    cast to bf16 (the BIR verifier requires both matmul operands to
    share a dtype; bf16 runs full rate, rel err ~2e-3 vs the 2e-2 gate)
  - degree 0 (T0 == 1) is folded via V-fold: V[i',o] = sum_k W0[i'+128k,o]
    is pre-reduced on the vector engine (7 adds, off the PE), and each
    half accumulates it with 8 mid-stream start=False matmuls
    V.T @ ones[128,512]; replaces 128 full matmuls (1/9 of PE work)
    with 16 and adds zero head-of-pipe serialization
  - contraction: stationary = W[d, i-tile, o-tile] bf16, moving =
    T_d[i-tile, b-half] bf16; psum holds y.T chunks
    [o-tile 128, b-half 512] x 8 o-tiles = 8 banks
  - two b-halves of 512; W streamed from HBM once per half (bf16)
  - the last degree of each half runs ot-major so psum banks close
    progressively: evacuation + output DMA overlap the matmul stream
  - output is y.T per core; host transposes and concatenates
"""

import numpy as np
import ml_dtypes

import concourse.bass as bass
import concourse.tile as tile
from concourse import bacc, mybir
from concourse import bass_utils
from concourse.alu_op_type import AluOpType

N_CORES = 8
B = 8192
IC = 1024
OC = 1024
DEG = 8  # polynomial degree; DEG+1 = 9 basis terms
BC = B // N_CORES  # 1024 batch rows per core
P = 128
NI = IC // P  # 8 i-tiles
NO = OC // P  # 8 o-tiles
BH = BC // 2  # 512, b-half
F32 = mybir.dt.float32
BF16 = mybir.dt.bfloat16

# W slab granularity (in i-tiles): first degree of a half uses small
# head slabs so its first matmul's W-DMA chain is short.
_D1_SLABS = [1, 1, 2, 2, 2]
_D_SLABS = [2, 2, 2, 2]


def _build(tanh_scale: float, tanh_bias: float):
    nc = bacc.Bacc("TRN2", target_bir_lowering=False, debug=False, num_devices=N_CORES)

    xT_d = nc.dram_tensor("xT", [IC, BC], F32, kind="ExternalInput").ap()
    w_d = nc.dram_tensor("w", [DEG + 1, IC, OC], BF16, kind="ExternalInput").ap()
    yt_d = nc.dram_tensor("yt", [OC, BC], F32, kind="ExternalOutput").ap()

    with tile.TileContext(nc) as tc:
        with (
            tc.tile_pool(name="const", bufs=1) as constp,
            tc.tile_pool(name="xin", bufs=5) as xinp,
            tc.tile_pool(name="xt", bufs=2) as xtp,
            tc.tile_pool(name="state", bufs=3) as statep,
            tc.tile_pool(name="prod", bufs=2) as prodp,
            tc.tile_pool(name="tr", bufs=3) as trp,
            tc.tile_pool(name="wstage", bufs=8) as wstagep,
            tc.tile_pool(name="evac", bufs=3) as evacp,
            tc.tile_pool(name="ps", bufs=8, space=bass.MemorySpace.PSUM) as psp,
        ):
            ones512 = constp.tile([P, BH], BF16, tag="ones")
            nc.vector.memset(ones512[:], 1.0)
            v_bf = constp.tile([P, OC], BF16, tag="vbf")  # V: d0 fold, bf16

            def emit_w_slabs(h, d, slab_sizes, it0=0, tag=None):
                """DMA W[d] i-tile slabs (bf16, used directly as stationary);
                returns [(first_it, ntiles, tile), ...]."""
                out = []
                for ws, nt in enumerate(slab_sizes):
                    wst = wstagep.tile(
                        [P, nt * OC],
                        BF16,
                        tag="wstage",
                        name=f"wst_{h}_{tag or d}_{ws}_{it0}",
                    )
                    nc.sync.dma_start(
                        wst[:].rearrange("p (il o) -> p il o", il=nt),
                        w_d[d, it0 * P : (it0 + nt) * P, :].rearrange(
                            "(il p) o -> p il o", p=P
                        ),
                    )
                    out.append((it0, nt, wst))
                    it0 += nt
                return out

            # ---- x.T chunk DMAs (batched up-front on a chosen queue) ----
            def emit_x_dmas(h, chunks, it0=0, eng=None):
                eng = eng or nc.sync
                tiles = []
                for nt in chunks:
                    xst = xinp.tile(
                        [P, nt * BH], F32, tag="xin", name=f"xs_{h}_{it0}"
                    )
                    eng.dma_start(
                        xst[:].rearrange("p (il b) -> p il b", il=nt),
                        xT_d[
                            it0 * P : (it0 + nt) * P, h * BH : (h + 1) * BH
                        ].rearrange("(il p) b -> p il b", p=P),
                    )
                    tiles.append((it0, nt, xst))
                    it0 += nt
                return tiles

            # ---- tanh (scalar) + T1 bf16 cast per staged chunk ----
            def emit_tanh_chunks(h, xt, tr1, xtiles, tr1_eng="vector"):
                for it0, nt, xst in xtiles:
                    sl = slice(it0 * BH, (it0 + nt) * BH)
                    nc.scalar.activation(
                        xt[:, sl],
                        xst[:],
                        mybir.ActivationFunctionType.Tanh,
                        bias=tanh_bias,
                        scale=tanh_scale,
                    )
                    if tr1_eng == "vector":
                        nc.vector.tensor_copy(tr1[:, sl], xt[:, sl])
                    else:
                        nc.scalar.activation(
                            tr1[:, sl], xt[:, sl], mybir.ActivationFunctionType.Copy
                        )

            def alloc_xt(h):
                xt = xtp.tile([P, NI * BH], F32, tag="xt", name=f"xt_{h}")
                tr1 = trp.tile([P, NI * BH], BF16, tag="tr1", name=f"tr_{h}_1", bufs=2)
                return xt, tr1

            def emit_matmuls(accs, wr_slabs, d, tr_d):
                if d == DEG:
                    # whole last degree ot-major: each acc's accumulation
                    # closes early so psum evac + y DMA overlap the stream
                    for ot in range(NO):
                        for si, (it0, nt, wst) in enumerate(wr_slabs):
                            for il in range(nt):
                                it = it0 + il
                                nc.tensor.matmul(
                                    accs[ot][:],
                                    wst[:, il * OC + ot * P : il * OC + (ot + 1) * P],
                                    tr_d[:, it * BH : (it + 1) * BH],
                                    start=False,
                                    stop=(it == NI - 1),
                                )
                    return
                for si, (it0, nt, wst) in enumerate(wr_slabs):
                    for il in range(nt):
                        it = it0 + il
                        rhs = tr_d[:, it * BH : (it + 1) * BH]
                        for ot in range(NO):
                            nc.tensor.matmul(
                                accs[ot][:],
                                wst[:, il * OC + ot * P : il * OC + (ot + 1) * P],
                                rhs,
                                start=(d == 1 and it == 0),
                                stop=False,
                            )

            def emit_d0(accs):
                # d0 fold: acc[ot] += V.T @ ones  (adds y0[o] to every b col)
                for ot in range(NO):
                    nc.tensor.matmul(
                        accs[ot][:],
                        v_bf[:, ot * P : (ot + 1) * P],
                        ones512[:],
                        start=False,
                        stop=False,
                    )

            # ---- head: x chunks on the scalar DMA queue (parallel to the
            # W stream on sync); all dma_starts queued before the tanh chain
            # so the scalar engine FIFO never blocks a transfer start ----
            xt0, tr1_0 = alloc_xt(0)
            x0_tiles = emit_x_dmas(0, chunks=[1, 1], eng=nc.sync)
            d1_head = emit_w_slabs(0, 1, _D1_SLABS[:2])
            x0_tiles += emit_x_dmas(0, chunks=[2, 2, 2], it0=2, eng=nc.scalar)
            emit_tanh_chunks(0, xt0, tr1_0, x0_tiles)

            xts = [(xt0, tr1_0), None]

            for h in range(2):
                if h == 0:
                    d1_slabs_h0 = d1_head + emit_w_slabs(
                        0, 1, _D1_SLABS[2:], it0=2, tag="1b"
                    )
                xt, tr1 = xts[h]
                # ---- accumulation psum tiles: y.T chunk per o-tile ----
                accs = [
                    psp.tile([P, BH], F32, tag="ps", name=f"acc_h{h}_o{ot}")
                    for ot in range(NO)
                ]

                # ---- degree loop (d0 folded mid-stream via emit_d0) ----
                t_m1 = xt  # T_{d-1} (fp32 slab)
                t_m2 = None  # T_{d-2}
                QS = NI * BH // 4
                for d in range(1, DEG + 1):
                    if d == 1:
                        tr_d = tr1
                    else:
                        tr_d = trp.tile(
                            [P, NI * BH], BF16, tag="tr", name=f"tr_{h}_{d}"
                        )
                        t_new = statep.tile(
                            [P, NI * BH], F32, tag="state", name=f"st_{h}_{d}"
                        )
                        for q in range(4):
                            sl = slice(q * QS, (q + 1) * QS)
                            prod = prodp.tile(
                                [P, QS], F32, tag="prod", name=f"prod_{h}_{d}_{q}"
                            )
                            nc.vector.scalar_tensor_tensor(
                                prod[:],
                                t_m1[:, sl],
                                2.0,
                                xt[:, sl],
                                AluOpType.mult,
                                AluOpType.mult,
                            )
                            if d == 2:
                                # T2 = 2*xt^2 - 1
                                nc.vector.tensor_scalar_sub(t_new[:, sl], prod[:], 1.0)
                            else:
                                nc.vector.tensor_sub(t_new[:, sl], prod[:], t_m2[:, sl])
                            if h == 1 and d == 2 and q < 2:
                                # boundary: scalar queue is draining h0's
                                # evacs; cast on DVE so d2's matmuls start
                                nc.vector.tensor_copy(tr_d[:, sl], t_new[:, sl])
                            else:
                                nc.scalar.activation(
                                    tr_d[:, sl],
                                    t_new[:, sl],
                                    mybir.ActivationFunctionType.Copy,
                                )
                        t_m2, t_m1 = t_m1, t_new

                    # ---- W stream + matmuls for this degree ----
                    if h == 0 and d == 1:
                        wr_slabs = d1_slabs_h0
                    elif h == 1 and d == 1:
                        wr_slabs = h1_d1_head + emit_w_slabs(
                            1, 1, _D1_SLABS[2:], it0=2, tag="1b"
                        )
                    else:
                        wr_slabs = emit_w_slabs(h, d, _D1_SLABS if d == 1 else _D_SLABS)
                    emit_matmuls(accs, wr_slabs, d, tr_d)

                    if h == 0 and d == 2:
                        xt1, tr1_1 = alloc_xt(1)
                        x1_tiles = emit_x_dmas(1, chunks=[2, 2, 2, 2], eng=nc.sync)
                        emit_tanh_chunks(1, xt1, tr1_1, x1_tiles, tr1_eng="scalar")
                        xts[1] = (xt1, tr1_1)
                    if h == 0 and d == 3:
                        # W0 stream for the V-fold, after d3's slabs (the DMA
                        # queue has plenty of slack mid-stream)
                        w0_slabs = emit_w_slabs(0, 0, _D_SLABS, tag="w0")
                    if h == 0 and d == 4:
                        # V = sum of W0's 8 i-tiles (DVE), then cast to bf16
                        vtmp = [
                            prodp.tile([P, OC], F32, tag="vtmp", name=f"v_{j}", bufs=2)
                            for j in range(2)
                        ]
                        w0_chunks = [
                            wst[:, il * OC : (il + 1) * OC]
                            for it0, nt, wst in w0_slabs
                            for il in range(nt)
                        ]
                        nc.vector.tensor_add(vtmp[0][:], w0_chunks[0], w0_chunks[1])
                        sel = 0
                        for k, ch in enumerate(w0_chunks[2:]):
                            sel = (k + 1) % 2
                            nc.vector.tensor_add(vtmp[sel][:], vtmp[k % 2][:], ch)
                        nc.vector.tensor_copy(v_bf[:], vtmp[sel][:])
                    if (h == 0 and d == 5) or (h == 1 and d == 2):
                        emit_d0(accs)
                    if h == 0 and d == 7:
                        # prefetch h1's first-degree head slabs so the
                        # boundary isn't gated on their DMA
                        h1_d1_head = emit_w_slabs(1, 1, _D1_SLABS[:2])

                # ---- evacuate psum -> SBUF -> y.T ----
                # All evacs on scalar: the DVE FIFO is recurrence-critical at
                # the half boundary and queued evac copies there stall h1's
                # psum banks. h0: y DMAs on the scalar queue (sync must keep
                # feeding h1's W stream). h1 (tail): split across both queues.
                for ot in range(NO):
                    ev = evacp.tile([P, BH], F32, tag="evac", name=f"ev_h{h}_o{ot}")
                    nc.scalar.activation(
                        ev[:], accs[ot][:], mybir.ActivationFunctionType.Copy
                    )
                    dma_eng = nc.sync if (h == 1 and ot % 2 == 0) else nc.scalar
                    dma_eng.dma_start(
                        yt_d[ot * P : (ot + 1) * P, h * BH : (h + 1) * BH],
                        ev[:],
                    )

    nc.compile()
    return nc


_CACHE: dict = {}


def make_in_maps(x, w):
    w_bf = np.ascontiguousarray(np.asarray(w, dtype=np.float32)).astype(
        ml_dtypes.bfloat16
    )
    return [
        {"xT": np.ascontiguousarray(x[c * BC : (c + 1) * BC].T), "w": w_bf}
        for c in range(N_CORES)
    ]


def kernel(x, cheby_coeffs, tanh_scale, tanh_bias):
    x = np.ascontiguousarray(np.asarray(x, dtype=np.float32))
    ts = float(np.asarray(tanh_scale))
    tb = float(np.asarray(tanh_bias))

    key = (ts, tb)
    if key not in _CACHE:
        _CACHE[key] = _build(ts, tb)
    nc = _CACHE[key]

    in_maps = make_in_maps(x, cheby_coeffs)
    res = bass_utils.run_bass_kernel_spmd(
        nc, in_maps, core_ids=list(range(N_CORES)), trace=False
    )

    y = np.empty((B, OC), dtype=np.float32)
    for c in range(N_CORES):
        y[c * BC : (c + 1) * BC, :] = res.results[c]["yt"].T
    return y


# revision 24
# speedup vs baseline: 1.0356x; 1.0021x over previous
"""ChebyKAN layer on 8 Trainium2 NeuronCores.

y = einsum('dbi,dio->bo', cheby_basis(tanh(x)), cheby_coeffs)

Strategy (per core, data-parallel over batch):
  - each core takes 1024 rows of x (8192/8) and the full coeffs
  - x arrives pre-transposed ([i, b] layout) from the host; tanh on the
    scalar engine
  - cheby_coeffs arrive as bf16 (host cast): halves the dominant W DMA
    stream (the W tensor is streamed once per batch-half) and the bf16
    stationary operand gets fast weight load on the PE
  - Chebyshev basis built on-the-fly in fp32 on the vector engine and
    cast to bf16 (the BIR verifier requires both matmul operands to
    share a dtype; bf16 runs full rate, rel err ~2e-3 vs the 2e-2 gate)
  - degree 0 (T0 == 1) is folded via V-fold: V[i',o] = sum_k W0[i'+128k,o]
    is pre-reduced on the vector engine (7 adds, off the PE), and each
    half accumulates it with 8 mid-stream start=False matmuls
    V.T @ ones[128,512]; replaces 128 full matmuls (1/9 of PE work)
    with 16 and adds zero head-of-pipe serialization
  - contraction: stationary = W[d, i-tile, o-tile] bf16, moving =
    T_d[i-tile, b-half] bf16; psum holds y.T chunks
    [o-tile 128, b-half 512] x 8 o-tiles = 8 banks
  - two b-halves of 512; W streamed from HBM once per half (bf16)
  - the last degree of each half runs ot-major so psum banks close
    progressively: evacuation + output DMA overlap the matmul stream
  - output is y.T per core; host transposes and concatenates
"""

import numpy as np
import ml_dtypes

import concourse.bass as bass
import concourse.tile as tile
from concourse import bacc, mybir
from concourse import bass_utils
from concourse.alu_op_type import AluOpType

N_CORES = 8
B = 8192
IC = 1024
OC = 1024
DEG = 8  # polynomial degree; DEG+1 = 9 basis terms
BC = B // N_CORES  # 1024 batch rows per core
P = 128
NI = IC // P  # 8 i-tiles
NO = OC // P  # 8 o-tiles
BH = BC // 2  # 512, b-half
F32 = mybir.dt.float32
BF16 = mybir.dt.bfloat16

# W slab granularity (in i-tiles): first degree of a half uses small
# head slabs so its first matmul's W-DMA chain is short.
_D1_SLABS = [1, 1, 2, 2, 2]
_D_SLABS = [2, 2, 2, 2]


def _build(tanh_scale: float, tanh_bias: float):
    nc = bacc.Bacc("TRN2", target_bir_lowering=False, debug=False, num_devices=N_CORES)

    xT_d = nc.dram_tensor("xT", [IC, BC], F32, kind="ExternalInput").ap()
    w_d = nc.dram_tensor("w", [DEG + 1, IC, OC], BF16, kind="ExternalInput").ap()
    yt_d = nc.dram_tensor("yt", [OC, BC], F32, kind="ExternalOutput").ap()

    with tile.TileContext(nc) as tc:
        with (
            tc.tile_pool(name="const", bufs=1) as constp,
            tc.tile_pool(name="xin", bufs=5) as xinp,
            tc.tile_pool(name="xt", bufs=2) as xtp,
            tc.tile_pool(name="state", bufs=3) as statep,
            tc.tile_pool(name="prod", bufs=2) as prodp,
            tc.tile_pool(name="tr", bufs=3) as trp,
            tc.tile_pool(name="wstage", bufs=8) as wstagep,
            tc.tile_pool(name="evac", bufs=3) as evacp,
            tc.tile_pool(name="ps", bufs=8, space=bass.MemorySpace.PSUM) as psp,
        ):
            ones512 = constp.tile([P, BH], BF16, tag="ones")
            nc.vector.memset(ones512[:], 1.0)
            v_bf = constp.tile([P, OC], BF16, tag="vbf")  # V: d0 fold, bf16

            def emit_w_slabs(h, d, slab_sizes, it0=0, tag=None):
                """DMA W[d] i-tile slabs (bf16, used directly as stationary);
                returns [(first_it, ntiles, tile), ...]."""
                out = []
                for ws, nt in enumerate(slab_sizes):
                    wst = wstagep.tile(
                        [P, nt * OC],
                        BF16,
                        tag="wstage",
                        name=f"wst_{h}_{tag or d}_{ws}_{it0}",
                    )
                    nc.sync.dma_start(
                        wst[:].rearrange("p (il o) -> p il o", il=nt),
                        w_d[d, it0 * P : (it0 + nt) * P, :].rearrange(
                            "(il p) o -> p il o", p=P
                        ),
                    )
                    out.append((it0, nt, wst))
                    it0 += nt
                return out

            # ---- x.T chunk DMAs (batched up-front on a chosen queue) ----
            def emit_x_dmas(h, chunks, it0=0, eng=None):
                eng = eng or nc.sync
                tiles = []
                for nt in chunks:
                    xst = xinp.tile(
                        [P, nt * BH], F32, tag="xin", name=f"xs_{h}_{it0}"
                    )
                    eng.dma_start(
                        xst[:].rearrange("p (il b) -> p il b", il=nt),
                        xT_d[
                            it0 * P : (it0 + nt) * P, h * BH : (h + 1) * BH
                        ].rearrange("(il p) b -> p il b", p=P),
                    )
                    tiles.append((it0, nt, xst))
                    it0 += nt
                return tiles

            # ---- tanh (scalar) + T1 bf16 cast per staged chunk ----
            def emit_tanh_chunks(h, xt, tr1, xtiles, tr1_eng="vector"):
                for it0, nt, xst in xtiles:
                    sl = slice(it0 * BH, (it0 + nt) * BH)
                    nc.scalar.activation(
                        xt[:, sl],
                        xst[:],
                        mybir.ActivationFunctionType.Tanh,
                        bias=tanh_bias,
                        scale=tanh_scale,
                    )
                    if tr1_eng == "vector":
                        nc.vector.tensor_copy(tr1[:, sl], xt[:, sl])
                    else:
                        nc.scalar.activation(
                            tr1[:, sl], xt[:, sl], mybir.ActivationFunctionType.Copy
                        )

            def alloc_xt(h):
                xt = xtp.tile([P, NI * BH], F32, tag="xt", name=f"xt_{h}")
                tr1 = trp.tile([P, NI * BH], BF16, tag="tr1", name=f"tr_{h}_1", bufs=2)
                return xt, tr1

            def emit_matmuls(accs, wr_slabs, d, tr_d):
                if d == DEG:
                    # whole last degree ot-major: each acc's accumulation
                    # closes early so psum evac + y DMA overlap the stream
                    for ot in range(NO):
                        for si, (it0, nt, wst) in enumerate(wr_slabs):
                            for il in range(nt):
                                it = it0 + il
                                nc.tensor.matmul(
                                    accs[ot][:],
                                    wst[:, il * OC + ot * P : il * OC + (ot + 1) * P],
                                    tr_d[:, it * BH : (it + 1) * BH],
                                    start=False,
                                    stop=(it == NI - 1),
                                )
                    return
                for si, (it0, nt, wst) in enumerate(wr_slabs):
                    for il in range(nt):
                        it = it0 + il
                        rhs = tr_d[:, it * BH : (it + 1) * BH]
                        for ot in range(NO):
                            nc.tensor.matmul(
                                accs[ot][:],
                                wst[:, il * OC + ot * P : il * OC + (ot + 1) * P],
                                rhs,
                                start=(d == 1 and it == 0),
                                stop=False,
                            )

            def emit_d0(accs):
                # d0 fold: acc[ot] += V.T @ ones  (adds y0[o] to every b col)
                for ot in range(NO):
                    nc.tensor.matmul(
                        accs[ot][:],
                        v_bf[:, ot * P : (ot + 1) * P],
                        ones512[:],
                        start=False,
                        stop=False,
                    )

            # ---- head: x chunks on the scalar DMA queue (parallel to the
            # W stream on sync); all dma_starts queued before the tanh chain
            # so the scalar engine FIFO never blocks a transfer start ----
            xt0, tr1_0 = alloc_xt(0)
            x0_tiles = emit_x_dmas(0, chunks=[1], eng=nc.sync)
            d1_head = emit_w_slabs(0, 1, _D1_SLABS[:2])
            x0_tiles += emit_x_dmas(0, chunks=[1], it0=1, eng=nc.sync)
            x0_tiles += emit_x_dmas(0, chunks=[2, 2, 2], it0=2, eng=nc.scalar)
            emit_tanh_chunks(0, xt0, tr1_0, x0_tiles)

            # PE warm-up: ~30 dummy matmuls on the ones tile, gated only on
            # the memset, so the HAM clock gate opens (1.2 -> 2.4 GHz)
            # before the first real matmul instead of ~3.4us into the stream
            wu = psp.tile([P, 64], F32, tag="ps", name="wu")
            for k in range(30):
                nc.tensor.matmul(
                    wu[:],
                    ones512[:, 0:P],
                    ones512[:, 0:64],
                    start=(k == 0),
                    stop=(k == 29),
                )

            xts = [(xt0, tr1_0), None]

            for h in range(2):
                if h == 0:
                    d1_slabs_h0 = d1_head + emit_w_slabs(
                        0, 1, _D1_SLABS[2:], it0=2, tag="1b"
                    )
                xt, tr1 = xts[h]
                # ---- accumulation psum tiles: y.T chunk per o-tile ----
                accs = [
                    psp.tile([P, BH], F32, tag="ps", name=f"acc_h{h}_o{ot}")
                    for ot in range(NO)
                ]

                # ---- degree loop (d0 folded mid-stream via emit_d0) ----
                t_m1 = xt  # T_{d-1} (fp32 slab)
                t_m2 = None  # T_{d-2}
                QS = NI * BH // 4
                for d in range(1, DEG + 1):
                    if d == 1:
                        tr_d = tr1
                    else:
                        tr_d = trp.tile(
                            [P, NI * BH], BF16, tag="tr", name=f"tr_{h}_{d}"
                        )
                        t_new = statep.tile(
                            [P, NI * BH], F32, tag="state", name=f"st_{h}_{d}"
                        )
                        for q in range(4):
                            sl = slice(q * QS, (q + 1) * QS)
                            prod = prodp.tile(
                                [P, QS], F32, tag="prod", name=f"prod_{h}_{d}_{q}"
                            )
                            nc.vector.scalar_tensor_tensor(
                                prod[:],
                                t_m1[:, sl],
                                2.0,
                                xt[:, sl],
                                AluOpType.mult,
                                AluOpType.mult,
                            )
                            if d == 2:
                                # T2 = 2*xt^2 - 1
                                nc.vector.tensor_scalar_sub(t_new[:, sl], prod[:], 1.0)
                            else:
                                nc.vector.tensor_sub(t_new[:, sl], prod[:], t_m2[:, sl])
                            if h == 1 and d == 2 and q < 2:
                                # boundary: scalar queue is draining h0's
                                # evacs; cast on DVE so d2's matmuls start
                                nc.vector.tensor_copy(tr_d[:, sl], t_new[:, sl])
                            else:
                                nc.scalar.activation(
                                    tr_d[:, sl],
                                    t_new[:, sl],
                                    mybir.ActivationFunctionType.Copy,
                                )
                        t_m2, t_m1 = t_m1, t_new

                    # ---- W stream + matmuls for this degree ----
                    if h == 0 and d == 1:
                        wr_slabs = d1_slabs_h0
                    elif h == 1 and d == 1:
                        wr_slabs = h1_d1_head + emit_w_slabs(
                            1, 1, _D1_SLABS[2:], it0=2, tag="1b"
                        )
                    else:
                        wr_slabs = emit_w_slabs(h, d, _D1_SLABS if d == 1 else _D_SLABS)
                    emit_matmuls(accs, wr_slabs, d, tr_d)

                    if h == 0 and d == 2:
                        xt1, tr1_1 = alloc_xt(1)
                        x1_tiles = emit_x_dmas(1, chunks=[2, 2, 2, 2], eng=nc.sync)
                        emit_tanh_chunks(1, xt1, tr1_1, x1_tiles, tr1_eng="scalar")
                        xts[1] = (xt1, tr1_1)
                    if h == 0 and d == 3:
                        # W0 stream for the V-fold, after d3's slabs (the DMA
                        # queue has plenty of slack mid-stream)
                        w0_slabs = emit_w_slabs(0, 0, _D_SLABS, tag="w0")
                    if h == 0 and d == 4:
                        # V = sum of W0's 8 i-tiles (DVE), then cast to bf16
                        vtmp = [
                            prodp.tile([P, OC], F32, tag="vtmp", name=f"v_{j}", bufs=2)
                            for j in range(2)
                        ]
                        w0_chunks = [
                            wst[:, il * OC : (il + 1) * OC]
                            for it0, nt, wst in w0_slabs
                            for il in range(nt)
                        ]
                        nc.vector.tensor_add(vtmp[0][:], w0_chunks[0], w0_chunks[1])
                        sel = 0
                        for k, ch in enumerate(w0_chunks[2:]):
                            sel = (k + 1) % 2
                            nc.vector.tensor_add(vtmp[sel][:], vtmp[k % 2][:], ch)
                        nc.vector.tensor_copy(v_bf[:], vtmp[sel][:])
                    if (h == 0 and d == 5) or (h == 1 and d == 2):
                        emit_d0(accs)
                    if h == 0 and d == 7:
                        # prefetch h1's first-degree head slabs so the
                        # boundary isn't gated on their DMA
                        h1_d1_head = emit_w_slabs(1, 1, _D1_SLABS[:2])

                # ---- evacuate psum -> SBUF -> y.T ----
                # All evacs on scalar: the DVE FIFO is recurrence-critical at
                # the half boundary and queued evac copies there stall h1's
                # psum banks. h0: y DMAs on the scalar queue (sync must keep
                # feeding h1's W stream). h1 (tail): split across both queues.
                for ot in range(NO):
                    ev = evacp.tile([P, BH], F32, tag="evac", name=f"ev_h{h}_o{ot}")
                    nc.scalar.activation(
                        ev[:], accs[ot][:], mybir.ActivationFunctionType.Copy
                    )
                    dma_eng = nc.sync if (h == 1 and ot % 2 == 0) else nc.scalar
                    dma_eng.dma_start(
                        yt_d[ot * P : (ot + 1) * P, h * BH : (h + 1) * BH],
                        ev[:],
                    )

    nc.compile()
    return nc


_CACHE: dict = {}


def make_in_maps(x, w):
    w_bf = np.ascontiguousarray(np.asarray(w, dtype=np.float32)).astype(
        ml_dtypes.bfloat16
    )
    return [
        {"xT": np.ascontiguousarray(x[c * BC : (c + 1) * BC].T), "w": w_bf}
        for c in range(N_CORES)
    ]


def kernel(x, cheby_coeffs, tanh_scale, tanh_bias):
    x = np.ascontiguousarray(np.asarray(x, dtype=np.float32))
    ts = float(np.asarray(tanh_scale))
    tb = float(np.asarray(tanh_bias))

    key = (ts, tb)
    if key not in _CACHE:
        _CACHE[key] = _build(ts, tb)
    nc = _CACHE[key]

    in_maps = make_in_maps(x, cheby_coeffs)
    res = bass_utils.run_bass_kernel_spmd(
        nc, in_maps, core_ids=list(range(N_CORES)), trace=False
    )

    y = np.empty((B, OC), dtype=np.float32)
    for c in range(N_CORES):
        y[c * BC : (c + 1) * BC, :] = res.results[c]["yt"].T
    return y


# revision 25
# speedup vs baseline: 1.0371x; 1.0014x over previous
"""ChebyKAN layer on 8 Trainium2 NeuronCores.

y = einsum('dbi,dio->bo', cheby_basis(tanh(x)), cheby_coeffs)

Strategy (per core, data-parallel over batch):
  - each core takes 1024 rows of x (8192/8) and the full coeffs
  - x arrives pre-transposed ([i, b] layout) from the host; tanh on the
    scalar engine
  - cheby_coeffs arrive as bf16 (host cast): halves the dominant W DMA
    stream (the W tensor is streamed once per batch-half) and the bf16
    stationary operand gets fast weight load on the PE
  - Chebyshev basis built on-the-fly in fp32 on the vector engine and
    cast to bf16 (the BIR verifier requires both matmul operands to
    share a dtype; bf16 runs full rate, rel err ~2e-3 vs the 2e-2 gate)
  - degree 0 (T0 == 1) is folded via V-fold: V[i',o] = sum_k W0[i'+128k,o]
    is pre-reduced on the vector engine (7 adds, off the PE), and each
    half accumulates it with 8 mid-stream start=False matmuls
    V.T @ ones[128,512]; replaces 128 full matmuls (1/9 of PE work)
    with 16 and adds zero head-of-pipe serialization
  - contraction: stationary = W[d, i-tile, o-tile] bf16, moving =
    T_d[i-tile, b-half] bf16; psum holds y.T chunks
    [o-tile 128, b-half 512] x 8 o-tiles = 8 banks
  - two b-halves of 512; W streamed from HBM once per half (bf16)
  - the last degree of each half runs ot-major so psum banks close
    progressively: evacuation + output DMA overlap the matmul stream
  - output is y.T per core; host transposes and concatenates
"""

import numpy as np
import ml_dtypes

import concourse.bass as bass
import concourse.tile as tile
from concourse import bacc, mybir
from concourse import bass_utils
from concourse.alu_op_type import AluOpType

N_CORES = 8
B = 8192
IC = 1024
OC = 1024
DEG = 8  # polynomial degree; DEG+1 = 9 basis terms
BC = B // N_CORES  # 1024 batch rows per core
P = 128
NI = IC // P  # 8 i-tiles
NO = OC // P  # 8 o-tiles
BH = BC // 2  # 512, b-half
F32 = mybir.dt.float32
BF16 = mybir.dt.bfloat16

# W slab granularity (in i-tiles): first degree of a half uses small
# head slabs so its first matmul's W-DMA chain is short.
_D1_SLABS = [1, 1, 2, 2, 2]
_D_SLABS = [2, 2, 2, 2]


def _build(tanh_scale: float, tanh_bias: float):
    nc = bacc.Bacc("TRN2", target_bir_lowering=False, debug=False, num_devices=N_CORES)

    xT_d = nc.dram_tensor("xT", [IC, BC], F32, kind="ExternalInput").ap()
    w_d = nc.dram_tensor("w", [DEG + 1, IC, OC], BF16, kind="ExternalInput").ap()
    yt_d = nc.dram_tensor("yt", [OC, BC], F32, kind="ExternalOutput").ap()

    with tile.TileContext(nc) as tc:
        with (
            tc.tile_pool(name="const", bufs=1) as constp,
            tc.tile_pool(name="xin", bufs=5) as xinp,
            tc.tile_pool(name="xt", bufs=2) as xtp,
            tc.tile_pool(name="state", bufs=3) as statep,
            tc.tile_pool(name="prod", bufs=2) as prodp,
            tc.tile_pool(name="tr", bufs=3) as trp,
            tc.tile_pool(name="wstage", bufs=8) as wstagep,
            tc.tile_pool(name="evac", bufs=3) as evacp,
            tc.tile_pool(name="ps", bufs=8, space=bass.MemorySpace.PSUM) as psp,
        ):
            ones512 = constp.tile([P, BH], BF16, tag="ones")
            nc.vector.memset(ones512[:], 1.0)
            v_bf = constp.tile([P, OC], BF16, tag="vbf")  # V: d0 fold, bf16

            def emit_w_slabs(h, d, slab_sizes, it0=0, tag=None):
                """DMA W[d] i-tile slabs (bf16, used directly as stationary);
                returns [(first_it, ntiles, tile), ...]."""
                out = []
                for ws, nt in enumerate(slab_sizes):
                    wst = wstagep.tile(
                        [P, nt * OC],
                        BF16,
                        tag="wstage",
                        name=f"wst_{h}_{tag or d}_{ws}_{it0}",
                    )
                    nc.sync.dma_start(
                        wst[:].rearrange("p (il o) -> p il o", il=nt),
                        w_d[d, it0 * P : (it0 + nt) * P, :].rearrange(
                            "(il p) o -> p il o", p=P
                        ),
                    )
                    out.append((it0, nt, wst))
                    it0 += nt
                return out

            # ---- x.T chunk DMAs (batched up-front on a chosen queue) ----
            def emit_x_dmas(h, chunks, it0=0, eng=None):
                eng = eng or nc.sync
                tiles = []
                for nt in chunks:
                    xst = xinp.tile(
                        [P, nt * BH], F32, tag="xin", name=f"xs_{h}_{it0}"
                    )
                    eng.dma_start(
                        xst[:].rearrange("p (il b) -> p il b", il=nt),
                        xT_d[
                            it0 * P : (it0 + nt) * P, h * BH : (h + 1) * BH
                        ].rearrange("(il p) b -> p il b", p=P),
                    )
                    tiles.append((it0, nt, xst))
                    it0 += nt
                return tiles

            # ---- tanh (scalar) + T1 bf16 cast per staged chunk ----
            def emit_tanh_chunks(h, xt, tr1, xtiles, tr1_eng="vector"):
                for it0, nt, xst in xtiles:
                    sl = slice(it0 * BH, (it0 + nt) * BH)
                    nc.scalar.activation(
                        xt[:, sl],
                        xst[:],
                        mybir.ActivationFunctionType.Tanh,
                        bias=tanh_bias,
                        scale=tanh_scale,
                    )
                    if tr1_eng == "vector":
                        nc.vector.tensor_copy(tr1[:, sl], xt[:, sl])
                    else:
                        nc.scalar.activation(
                            tr1[:, sl], xt[:, sl], mybir.ActivationFunctionType.Copy
                        )

            def alloc_xt(h):
                xt = xtp.tile([P, NI * BH], F32, tag="xt", name=f"xt_{h}")
                tr1 = trp.tile([P, NI * BH], BF16, tag="tr1", name=f"tr_{h}_1", bufs=2)
                return xt, tr1

            def emit_matmuls(accs, wr_slabs, d, tr_d):
                if d == DEG:
                    # whole last degree ot-major: each acc's accumulation
                    # closes early so psum evac + y DMA overlap the stream
                    for ot in range(NO):
                        for si, (it0, nt, wst) in enumerate(wr_slabs):
                            for il in range(nt):
                                it = it0 + il
                                nc.tensor.matmul(
                                    accs[ot][:],
                                    wst[:, il * OC + ot * P : il * OC + (ot + 1) * P],
                                    tr_d[:, it * BH : (it + 1) * BH],
                                    start=False,
                                    stop=(it == NI - 1),
                                )
                    return
                for si, (it0, nt, wst) in enumerate(wr_slabs):
                    for il in range(nt):
                        it = it0 + il
                        rhs = tr_d[:, it * BH : (it + 1) * BH]
                        for ot in range(NO):
                            nc.tensor.matmul(
                                accs[ot][:],
                                wst[:, il * OC + ot * P : il * OC + (ot + 1) * P],
                                rhs,
                                start=(d == 1 and it == 0),
                                stop=False,
                            )

            def emit_d0(accs):
                # d0 fold: acc[ot] += V.T @ ones  (adds y0[o] to every b col)
                for ot in range(NO):
                    nc.tensor.matmul(
                        accs[ot][:],
                        v_bf[:, ot * P : (ot + 1) * P],
                        ones512[:],
                        start=False,
                        stop=False,
                    )

            # ---- head: x chunks on the scalar DMA queue (parallel to the
            # W stream on sync); all dma_starts queued before the tanh chain
            # so the scalar engine FIFO never blocks a transfer start ----
            xt0, tr1_0 = alloc_xt(0)
            x0_tiles = emit_x_dmas(0, chunks=[1], eng=nc.sync)
            d1_head = emit_w_slabs(0, 1, _D1_SLABS[:2])
            x0_tiles += emit_x_dmas(0, chunks=[1, 2], it0=1, eng=nc.sync)
            x0_tiles += emit_x_dmas(0, chunks=[2, 2], it0=4, eng=nc.scalar)
            emit_tanh_chunks(0, xt0, tr1_0, x0_tiles)

            # PE warm-up: dummy matmuls on the ones tile, gated only on the
            # memset, sized to bridge until the first real matmul (~13us) so
            # the HAM clock gate opens (1.2 -> 2.4 GHz) with no idle window
            # in between — the real stream then starts at full clock
            wu = psp.tile([P, BH], F32, tag="ps", name="wu")
            for k in range(18):
                nc.tensor.matmul(
                    wu[:],
                    ones512[:, 0:P],
                    ones512[:],
                    start=(k == 0),
                    stop=(k == 17),
                )

            xts = [(xt0, tr1_0), None]

            for h in range(2):
                if h == 0:
                    d1_slabs_h0 = d1_head + emit_w_slabs(
                        0, 1, _D1_SLABS[2:], it0=2, tag="1b"
                    )
                xt, tr1 = xts[h]
                # ---- accumulation psum tiles: y.T chunk per o-tile ----
                accs = [
                    psp.tile([P, BH], F32, tag="ps", name=f"acc_h{h}_o{ot}")
                    for ot in range(NO)
                ]

                # ---- degree loop (d0 folded mid-stream via emit_d0) ----
                t_m1 = xt  # T_{d-1} (fp32 slab)
                t_m2 = None  # T_{d-2}
                QS = NI * BH // 4
                for d in range(1, DEG + 1):
                    if d == 1:
                        tr_d = tr1
                    else:
                        tr_d = trp.tile(
                            [P, NI * BH], BF16, tag="tr", name=f"tr_{h}_{d}"
                        )
                        t_new = statep.tile(
                            [P, NI * BH], F32, tag="state", name=f"st_{h}_{d}"
                        )
                        for q in range(4):
                            sl = slice(q * QS, (q + 1) * QS)
                            prod = prodp.tile(
                                [P, QS], F32, tag="prod", name=f"prod_{h}_{d}_{q}"
                            )
                            nc.vector.scalar_tensor_tensor(
                                prod[:],
                                t_m1[:, sl],
                                2.0,
                                xt[:, sl],
                                AluOpType.mult,
                                AluOpType.mult,
                            )
                            if d == 2:
                                # T2 = 2*xt^2 - 1
                                nc.vector.tensor_scalar_sub(t_new[:, sl], prod[:], 1.0)
                            else:
                                nc.vector.tensor_sub(t_new[:, sl], prod[:], t_m2[:, sl])
                            if h == 1 and d == 2 and q < 2:
                                # boundary: scalar queue is draining h0's
                                # evacs; cast on DVE so d2's matmuls start
                                nc.vector.tensor_copy(tr_d[:, sl], t_new[:, sl])
                            else:
                                nc.scalar.activation(
                                    tr_d[:, sl],
                                    t_new[:, sl],
                                    mybir.ActivationFunctionType.Copy,
                                )
                        t_m2, t_m1 = t_m1, t_new

                    # ---- W stream + matmuls for this degree ----
                    if h == 0 and d == 1:
                        wr_slabs = d1_slabs_h0
                    elif h == 1 and d == 1:
                        wr_slabs = h1_d1_head + emit_w_slabs(
                            1, 1, _D1_SLABS[2:], it0=2, tag="1b"
                        )
                    else:
                        wr_slabs = emit_w_slabs(h, d, _D1_SLABS if d == 1 else _D_SLABS)
                    emit_matmuls(accs, wr_slabs, d, tr_d)

                    if h == 0 and d == 2:
                        xt1, tr1_1 = alloc_xt(1)
                        x1_tiles = emit_x_dmas(1, chunks=[2, 2, 2, 2], eng=nc.sync)
                        emit_tanh_chunks(1, xt1, tr1_1, x1_tiles, tr1_eng="scalar")
                        xts[1] = (xt1, tr1_1)
                    if h == 0 and d == 3:
                        # W0 stream for the V-fold, after d3's slabs (the DMA
                        # queue has plenty of slack mid-stream)
                        w0_slabs = emit_w_slabs(0, 0, _D_SLABS, tag="w0")
                    if h == 0 and d == 4:
                        # V = sum of W0's 8 i-tiles (DVE), then cast to bf16
                        vtmp = [
                            prodp.tile([P, OC], F32, tag="vtmp", name=f"v_{j}", bufs=2)
                            for j in range(2)
                        ]
                        w0_chunks = [
                            wst[:, il * OC : (il + 1) * OC]
                            for it0, nt, wst in w0_slabs
                            for il in range(nt)
                        ]
                        nc.vector.tensor_add(vtmp[0][:], w0_chunks[0], w0_chunks[1])
                        sel = 0
                        for k, ch in enumerate(w0_chunks[2:]):
                            sel = (k + 1) % 2
                            nc.vector.tensor_add(vtmp[sel][:], vtmp[k % 2][:], ch)
                        nc.vector.tensor_copy(v_bf[:], vtmp[sel][:])
                    if (h == 0 and d == 5) or (h == 1 and d == 2):
                        emit_d0(accs)
                    if h == 0 and d == 7:
                        # prefetch h1's first-degree head slabs so the
                        # boundary isn't gated on their DMA
                        h1_d1_head = emit_w_slabs(1, 1, _D1_SLABS[:2])

                # ---- evacuate psum -> SBUF -> y.T ----
                # All evacs on scalar: the DVE FIFO is recurrence-critical at
                # the half boundary and queued evac copies there stall h1's
                # psum banks. h0: y DMAs on the scalar queue (sync must keep
                # feeding h1's W stream). h1 (tail): split across both queues.
                for ot in range(NO):
                    ev = evacp.tile([P, BH], F32, tag="evac", name=f"ev_h{h}_o{ot}")
                    nc.scalar.activation(
                        ev[:], accs[ot][:], mybir.ActivationFunctionType.Copy
                    )
                    dma_eng = nc.sync if (h == 1 and ot % 2 == 0) else nc.scalar
                    dma_eng.dma_start(
                        yt_d[ot * P : (ot + 1) * P, h * BH : (h + 1) * BH],
                        ev[:],
                    )

    nc.compile()
    return nc


_CACHE: dict = {}


def make_in_maps(x, w):
    w_bf = np.ascontiguousarray(np.asarray(w, dtype=np.float32)).astype(
        ml_dtypes.bfloat16
    )
    return [
        {"xT": np.ascontiguousarray(x[c * BC : (c + 1) * BC].T), "w": w_bf}
        for c in range(N_CORES)
    ]


def kernel(x, cheby_coeffs, tanh_scale, tanh_bias):
    x = np.ascontiguousarray(np.asarray(x, dtype=np.float32))
    ts = float(np.asarray(tanh_scale))
    tb = float(np.asarray(tanh_bias))

    key = (ts, tb)
    if key not in _CACHE:
        _CACHE[key] = _build(ts, tb)
    nc = _CACHE[key]

    in_maps = make_in_maps(x, cheby_coeffs)
    res = bass_utils.run_bass_kernel_spmd(
        nc, in_maps, core_ids=list(range(N_CORES)), trace=False
    )

    y = np.empty((B, OC), dtype=np.float32)
    for c in range(N_CORES):
        y[c * BC : (c + 1) * BC, :] = res.results[c]["yt"].T
    return y


# revision 26
# speedup vs baseline: 1.0392x; 1.0020x over previous
"""ChebyKAN layer on 8 Trainium2 NeuronCores.

y = einsum('dbi,dio->bo', cheby_basis(tanh(x)), cheby_coeffs)

Strategy (per core, data-parallel over batch):
  - each core takes 1024 rows of x (8192/8) and the full coeffs
  - x arrives pre-transposed ([i, b] layout) from the host; tanh on the
    scalar engine
  - cheby_coeffs arrive as bf16 (host cast): halves the dominant W DMA
    stream (the W tensor is streamed once per batch-half) and the bf16
    stationary operand gets fast weight load on the PE
  - Chebyshev basis built on-the-fly in fp32 on the vector engine and
    cast to bf16 (the BIR verifier requires both matmul operands to
    share a dtype; bf16 runs full rate, rel err ~2e-3 vs the 2e-2 gate)
  - degree 0 (T0 == 1) is folded via V-fold: V[i',o] = sum_k W0[i'+128k,o]
    is pre-reduced on the vector engine (7 adds, off the PE), and each
    half accumulates it with 8 mid-stream start=False matmuls
    V.T @ ones[128,512]; replaces 128 full matmuls (1/9 of PE work)
    with 16 and adds zero head-of-pipe serialization
  - contraction: stationary = W[d, i-tile, o-tile] bf16, moving =
    T_d[i-tile, b-half] bf16; psum holds y.T chunks
    [o-tile 128, b-half 512] x 8 o-tiles = 8 banks
  - two b-halves of 512; W streamed from HBM once per half (bf16)
  - the last degree of each half runs ot-major so psum banks close
    progressively: evacuation + output DMA overlap the matmul stream
  - output is y.T per core; host transposes and concatenates
"""

import numpy as np
import ml_dtypes

import concourse.bass as bass
import concourse.tile as tile
from concourse import bacc, mybir
from concourse import bass_utils
from concourse.alu_op_type import AluOpType

N_CORES = 8
B = 8192
IC = 1024
OC = 1024
DEG = 8  # polynomial degree; DEG+1 = 9 basis terms
BC = B // N_CORES  # 1024 batch rows per core
P = 128
NI = IC // P  # 8 i-tiles
NO = OC // P  # 8 o-tiles
BH = BC // 2  # 512, b-half
F32 = mybir.dt.float32
BF16 = mybir.dt.bfloat16

# W slab granularity (in i-tiles): first degree of a half uses small
# head slabs so its first matmul's W-DMA chain is short.
_D1_SLABS = [1, 1, 2, 2, 2]
_D_SLABS = [2, 2, 2, 2]


def _build(tanh_scale: float, tanh_bias: float):
    nc = bacc.Bacc("TRN2", target_bir_lowering=False, debug=False, num_devices=N_CORES)

    xT_d = nc.dram_tensor("xT", [IC, BC], F32, kind="ExternalInput").ap()
    w_d = nc.dram_tensor("w", [DEG + 1, IC, OC], BF16, kind="ExternalInput").ap()
    yt_d = nc.dram_tensor("yt", [OC, BC], F32, kind="ExternalOutput").ap()

    with tile.TileContext(nc) as tc:
        with (
            tc.tile_pool(name="const", bufs=1) as constp,
            tc.tile_pool(name="xin", bufs=5) as xinp,
            tc.tile_pool(name="xt", bufs=2) as xtp,
            tc.tile_pool(name="state", bufs=3) as statep,
            tc.tile_pool(name="prod", bufs=2) as prodp,
            tc.tile_pool(name="tr", bufs=3) as trp,
            tc.tile_pool(name="wstage", bufs=8) as wstagep,
            tc.tile_pool(name="evac", bufs=3) as evacp,
            tc.tile_pool(name="ps", bufs=8, space=bass.MemorySpace.PSUM) as psp,
        ):
            ones512 = constp.tile([P, BH], BF16, tag="ones")
            nc.vector.memset(ones512[:], 1.0)
            v_bf = constp.tile([P, OC], BF16, tag="vbf")  # V: d0 fold, bf16

            def emit_w_slabs(h, d, slab_sizes, it0=0, tag=None):
                """DMA W[d] i-tile slabs (bf16, used directly as stationary);
                returns [(first_it, ntiles, tile), ...]."""
                out = []
                for ws, nt in enumerate(slab_sizes):
                    wst = wstagep.tile(
                        [P, nt * OC],
                        BF16,
                        tag="wstage",
                        name=f"wst_{h}_{tag or d}_{ws}_{it0}",
                    )
                    nc.sync.dma_start(
                        wst[:].rearrange("p (il o) -> p il o", il=nt),
                        w_d[d, it0 * P : (it0 + nt) * P, :].rearrange(
                            "(il p) o -> p il o", p=P
                        ),
                    )
                    out.append((it0, nt, wst))
                    it0 += nt
                return out

            # ---- x.T chunk DMAs (batched up-front on a chosen queue) ----
            def emit_x_dmas(h, chunks, it0=0, eng=None):
                eng = eng or nc.sync
                tiles = []
                for nt in chunks:
                    xst = xinp.tile(
                        [P, nt * BH], F32, tag="xin", name=f"xs_{h}_{it0}"
                    )
                    eng.dma_start(
                        xst[:].rearrange("p (il b) -> p il b", il=nt),
                        xT_d[
                            it0 * P : (it0 + nt) * P, h * BH : (h + 1) * BH
                        ].rearrange("(il p) b -> p il b", p=P),
                    )
                    tiles.append((it0, nt, xst))
                    it0 += nt
                return tiles

            # ---- tanh (scalar) + T1 bf16 cast per staged chunk ----
            def emit_tanh_chunks(h, xt, tr1, xtiles, tr1_eng="vector"):
                for it0, nt, xst in xtiles:
                    sl = slice(it0 * BH, (it0 + nt) * BH)
                    nc.scalar.activation(
                        xt[:, sl],
                        xst[:],
                        mybir.ActivationFunctionType.Tanh,
                        bias=tanh_bias,
                        scale=tanh_scale,
                    )
                    if tr1_eng == "vector":
                        nc.vector.tensor_copy(tr1[:, sl], xt[:, sl])
                    else:
                        nc.scalar.activation(
                            tr1[:, sl], xt[:, sl], mybir.ActivationFunctionType.Copy
                        )

            def alloc_xt(h):
                xt = xtp.tile([P, NI * BH], F32, tag="xt", name=f"xt_{h}")
                tr1 = trp.tile([P, NI * BH], BF16, tag="tr1", name=f"tr_{h}_1", bufs=2)
                return xt, tr1

            def emit_matmuls(accs, wr_slabs, d, tr_d):
                if d == DEG:
                    # whole last degree ot-major: each acc's accumulation
                    # closes early so psum evac + y DMA overlap the stream
                    for ot in range(NO):
                        for si, (it0, nt, wst) in enumerate(wr_slabs):
                            for il in range(nt):
                                it = it0 + il
                                nc.tensor.matmul(
                                    accs[ot][:],
                                    wst[:, il * OC + ot * P : il * OC + (ot + 1) * P],
                                    tr_d[:, it * BH : (it + 1) * BH],
                                    start=False,
                                    stop=(it == NI - 1),
                                )
                    return
                for si, (it0, nt, wst) in enumerate(wr_slabs):
                    for il in range(nt):
                        it = it0 + il
                        rhs = tr_d[:, it * BH : (it + 1) * BH]
                        for ot in range(NO):
                            nc.tensor.matmul(
                                accs[ot][:],
                                wst[:, il * OC + ot * P : il * OC + (ot + 1) * P],
                                rhs,
                                start=(d == 1 and it == 0),
                                stop=False,
                            )

            def emit_d0(accs):
                # d0 fold: acc[ot] += V.T @ ones  (adds y0[o] to every b col)
                for ot in range(NO):
                    nc.tensor.matmul(
                        accs[ot][:],
                        v_bf[:, ot * P : (ot + 1) * P],
                        ones512[:],
                        start=False,
                        stop=False,
                    )

            # ---- head: x chunks on the scalar DMA queue (parallel to the
            # W stream on sync); all dma_starts queued before the tanh chain
            # so the scalar engine FIFO never blocks a transfer start ----
            xt0, tr1_0 = alloc_xt(0)
            x0_tiles = emit_x_dmas(0, chunks=[1], eng=nc.sync)
            d1_head = emit_w_slabs(0, 1, _D1_SLABS[:2])
            x0_tiles += emit_x_dmas(0, chunks=[1], it0=1, eng=nc.sync)
            x0_tiles += emit_x_dmas(0, chunks=[2, 2, 2], it0=2, eng=nc.scalar)
            emit_tanh_chunks(0, xt0, tr1_0, x0_tiles)

            # PE warm-up: dummy matmuls on the ones tile, gated only on the
            # memset, bridging until the first real matmul (~12us) so the
            # HAM clock gate opens (1.2 -> 2.4 GHz) with no idle window in
            # between — the real stream then starts at full clock. Two
            # alternating psum banks so fill/drain pipeline back-to-back.
            wus = [psp.tile([P, BH], F32, tag="ps", name=f"wu{j}") for j in range(2)]
            for k in range(20):
                nc.tensor.matmul(
                    wus[k % 2][:],
                    ones512[:, 0:P],
                    ones512[:],
                    start=(k < 2),
                    stop=(k >= 18),
                )

            xts = [(xt0, tr1_0), None]

            for h in range(2):
                if h == 0:
                    d1_slabs_h0 = d1_head + emit_w_slabs(
                        0, 1, _D1_SLABS[2:], it0=2, tag="1b"
                    )
                xt, tr1 = xts[h]
                # ---- accumulation psum tiles: y.T chunk per o-tile ----
                accs = [
                    psp.tile([P, BH], F32, tag="ps", name=f"acc_h{h}_o{ot}")
                    for ot in range(NO)
                ]

                # ---- degree loop (d0 folded mid-stream via emit_d0) ----
                t_m1 = xt  # T_{d-1} (fp32 slab)
                t_m2 = None  # T_{d-2}
                QS = NI * BH // 4
                for d in range(1, DEG + 1):
                    if d == 1:
                        tr_d = tr1
                    else:
                        tr_d = trp.tile(
                            [P, NI * BH], BF16, tag="tr", name=f"tr_{h}_{d}"
                        )
                        t_new = statep.tile(
                            [P, NI * BH], F32, tag="state", name=f"st_{h}_{d}"
                        )
                        for q in range(4):
                            sl = slice(q * QS, (q + 1) * QS)
                            prod = prodp.tile(
                                [P, QS], F32, tag="prod", name=f"prod_{h}_{d}_{q}"
                            )
                            nc.vector.scalar_tensor_tensor(
                                prod[:],
                                t_m1[:, sl],
                                2.0,
                                xt[:, sl],
                                AluOpType.mult,
                                AluOpType.mult,
                            )
                            if d == 2:
                                # T2 = 2*xt^2 - 1
                                nc.vector.tensor_scalar_sub(t_new[:, sl], prod[:], 1.0)
                            else:
                                nc.vector.tensor_sub(t_new[:, sl], prod[:], t_m2[:, sl])
                            if h == 1 and d == 2 and q < 2:
                                # boundary: scalar queue is draining h0's
                                # evacs; cast on DVE so d2's matmuls start
                                nc.vector.tensor_copy(tr_d[:, sl], t_new[:, sl])
                            else:
                                nc.scalar.activation(
                                    tr_d[:, sl],
                                    t_new[:, sl],
                                    mybir.ActivationFunctionType.Copy,
                                )
                        t_m2, t_m1 = t_m1, t_new

                    # ---- W stream + matmuls for this degree ----
                    if h == 0 and d == 1:
                        wr_slabs = d1_slabs_h0
                    elif h == 1 and d == 1:
                        wr_slabs = h1_d1_head + emit_w_slabs(
                            1, 1, _D1_SLABS[2:], it0=2, tag="1b"
                        )
                    else:
                        wr_slabs = emit_w_slabs(h, d, _D1_SLABS if d == 1 else _D_SLABS)
                    emit_matmuls(accs, wr_slabs, d, tr_d)

                    if h == 0 and d == 2:
                        xt1, tr1_1 = alloc_xt(1)
                        x1_tiles = emit_x_dmas(1, chunks=[2, 2, 2, 2], eng=nc.sync)
                        emit_tanh_chunks(1, xt1, tr1_1, x1_tiles, tr1_eng="scalar")
                        xts[1] = (xt1, tr1_1)
                    if h == 0 and d == 3:
                        # W0 stream for the V-fold, after d3's slabs (the DMA
                        # queue has plenty of slack mid-stream)
                        w0_slabs = emit_w_slabs(0, 0, _D_SLABS, tag="w0")
                    if h == 0 and d == 4:
                        # V = sum of W0's 8 i-tiles (DVE), then cast to bf16
                        vtmp = [
                            prodp.tile([P, OC], F32, tag="vtmp", name=f"v_{j}", bufs=2)
                            for j in range(2)
                        ]
                        w0_chunks = [
                            wst[:, il * OC : (il + 1) * OC]
                            for it0, nt, wst in w0_slabs
                            for il in range(nt)
                        ]
                        nc.vector.tensor_add(vtmp[0][:], w0_chunks[0], w0_chunks[1])
                        sel = 0
                        for k, ch in enumerate(w0_chunks[2:]):
                            sel = (k + 1) % 2
                            nc.vector.tensor_add(vtmp[sel][:], vtmp[k % 2][:], ch)
                        nc.vector.tensor_copy(v_bf[:], vtmp[sel][:])
                    if (h == 0 and d == 5) or (h == 1 and d == 2):
                        emit_d0(accs)
                    if h == 0 and d == 7:
                        # prefetch h1's first-degree head slabs so the
                        # boundary isn't gated on their DMA
                        h1_d1_head = emit_w_slabs(1, 1, _D1_SLABS[:2])

                # ---- evacuate psum -> SBUF -> y.T ----
                # All evacs on scalar: the DVE FIFO is recurrence-critical at
                # the half boundary and queued evac copies there stall h1's
                # psum banks. h0: y DMAs on the scalar queue (sync must keep
                # feeding h1's W stream). h1 (tail): split across both queues.
                for ot in range(NO):
                    ev = evacp.tile([P, BH], F32, tag="evac", name=f"ev_h{h}_o{ot}")
                    nc.scalar.activation(
                        ev[:], accs[ot][:], mybir.ActivationFunctionType.Copy
                    )
                    dma_eng = nc.sync if (h == 1 and ot % 2 == 0) else nc.scalar
                    dma_eng.dma_start(
                        yt_d[ot * P : (ot + 1) * P, h * BH : (h + 1) * BH],
                        ev[:],
                    )

    nc.compile()
    return nc


_CACHE: dict = {}


def make_in_maps(x, w):
    w_bf = np.ascontiguousarray(np.asarray(w, dtype=np.float32)).astype(
        ml_dtypes.bfloat16
    )
    return [
        {"xT": np.ascontiguousarray(x[c * BC : (c + 1) * BC].T), "w": w_bf}
        for c in range(N_CORES)
    ]


def kernel(x, cheby_coeffs, tanh_scale, tanh_bias):
    x = np.ascontiguousarray(np.asarray(x, dtype=np.float32))
    ts = float(np.asarray(tanh_scale))
    tb = float(np.asarray(tanh_bias))

    key = (ts, tb)
    if key not in _CACHE:
        _CACHE[key] = _build(ts, tb)
    nc = _CACHE[key]

    in_maps = make_in_maps(x, cheby_coeffs)
    res = bass_utils.run_bass_kernel_spmd(
        nc, in_maps, core_ids=list(range(N_CORES)), trace=False
    )

    y = np.empty((B, OC), dtype=np.float32)
    for c in range(N_CORES):
        y[c * BC : (c + 1) * BC, :] = res.results[c]["yt"].T
    return y


# revision 28
# speedup vs baseline: 1.0432x; 1.0039x over previous
"""ChebyKAN layer on 8 Trainium2 NeuronCores.

y = einsum('dbi,dio->bo', cheby_basis(tanh(x)), cheby_coeffs)

Strategy (per core, data-parallel over batch):
  - each core takes 1024 rows of x (8192/8) and the full coeffs
  - x arrives pre-transposed ([i, b] layout) from the host; tanh on the
    scalar engine
  - cheby_coeffs arrive as bf16 (host cast): halves the dominant W DMA
    stream (the W tensor is streamed once per batch-half) and the bf16
    stationary operand gets fast weight load on the PE
  - Chebyshev basis built on-the-fly in fp32 on the vector engine and
    cast to bf16 (the BIR verifier requires both matmul operands to
    share a dtype; bf16 runs full rate, rel err ~2e-3 vs the 2e-2 gate)
  - degree 0 (T0 == 1) is folded via V-fold: V[i',o] = sum_k W0[i'+128k,o]
    is pre-reduced on the vector engine (7 adds, off the PE), and each
    half accumulates it with 8 mid-stream start=False matmuls
    V.T @ ones[128,512]; replaces 128 full matmuls (1/9 of PE work)
    with 16 and adds zero head-of-pipe serialization
  - contraction: stationary = W[d, i-tile, o-tile] bf16, moving =
    T_d[i-tile, b-half] bf16; psum holds y.T chunks
    [o-tile 128, b-half 512] x 8 o-tiles = 8 banks
  - two b-halves of 512; W streamed from HBM once per half (bf16)
  - the last degree of each half runs ot-major so psum banks close
    progressively: evacuation + output DMA overlap the matmul stream
  - output is y.T per core; host transposes and concatenates
"""

import numpy as np
import ml_dtypes

import concourse.bass as bass
import concourse.tile as tile
from concourse import bacc, mybir
from concourse import bass_utils
from concourse.alu_op_type import AluOpType

N_CORES = 8
B = 8192
IC = 1024
OC = 1024
DEG = 8  # polynomial degree; DEG+1 = 9 basis terms
BC = B // N_CORES  # 1024 batch rows per core
P = 128
NI = IC // P  # 8 i-tiles
NO = OC // P  # 8 o-tiles
BH = BC // 2  # 512, b-half
F32 = mybir.dt.float32
BF16 = mybir.dt.bfloat16

# W slab granularity (in i-tiles): first degree of a half uses small
# head slabs so its first matmul's W-DMA chain is short.
_D1_SLABS = [1, 1, 2, 2, 2]
_D_SLABS = [2, 2, 2, 2]


def _build(tanh_scale: float, tanh_bias: float):
    nc = bacc.Bacc("TRN2", target_bir_lowering=False, debug=False, num_devices=N_CORES)

    xT_d = nc.dram_tensor("xT", [IC, BC], F32, kind="ExternalInput").ap()
    w_d = nc.dram_tensor("w", [DEG + 1, IC, OC], BF16, kind="ExternalInput").ap()
    yt_d = nc.dram_tensor("yt", [OC, BC], F32, kind="ExternalOutput").ap()

    with tile.TileContext(nc) as tc:
        with (
            tc.tile_pool(name="const", bufs=1) as constp,
            tc.tile_pool(name="xin", bufs=5) as xinp,
            tc.tile_pool(name="xt", bufs=2) as xtp,
            tc.tile_pool(name="state", bufs=3) as statep,
            tc.tile_pool(name="prod", bufs=2) as prodp,
            tc.tile_pool(name="tr", bufs=3) as trp,
            tc.tile_pool(name="wstage", bufs=8) as wstagep,
            tc.tile_pool(name="evac", bufs=3) as evacp,
            tc.tile_pool(name="ps", bufs=8, space=bass.MemorySpace.PSUM) as psp,
        ):
            ones512 = constp.tile([P, BH], BF16, tag="ones")
            nc.vector.memset(ones512[:], 1.0)
            v_bf = constp.tile([P, OC], BF16, tag="vbf")  # V: d0 fold, bf16

            def emit_w_slabs(h, d, slab_sizes, it0=0, tag=None):
                """DMA W[d] i-tile slabs (bf16, used directly as stationary);
                returns [(first_it, ntiles, tile), ...]."""
                out = []
                for ws, nt in enumerate(slab_sizes):
                    wst = wstagep.tile(
                        [P, nt * OC],
                        BF16,
                        tag="wstage",
                        name=f"wst_{h}_{tag or d}_{ws}_{it0}",
                    )
                    nc.sync.dma_start(
                        wst[:].rearrange("p (il o) -> p il o", il=nt),
                        w_d[d, it0 * P : (it0 + nt) * P, :].rearrange(
                            "(il p) o -> p il o", p=P
                        ),
                    )
                    out.append((it0, nt, wst))
                    it0 += nt
                return out

            # ---- x.T chunk DMAs (batched up-front on a chosen queue) ----
            def emit_x_dmas(h, chunks, it0=0, eng=None):
                eng = eng or nc.sync
                tiles = []
                for nt in chunks:
                    xst = xinp.tile(
                        [P, nt * BH], F32, tag="xin", name=f"xs_{h}_{it0}"
                    )
                    eng.dma_start(
                        xst[:].rearrange("p (il b) -> p il b", il=nt),
                        xT_d[
                            it0 * P : (it0 + nt) * P, h * BH : (h + 1) * BH
                        ].rearrange("(il p) b -> p il b", p=P),
                    )
                    tiles.append((it0, nt, xst))
                    it0 += nt
                return tiles

            # ---- tanh (scalar) + T1 bf16 cast per staged chunk ----
            def emit_tanh_chunks(h, xt, tr1, xtiles, tr1_eng="vector"):
                for it0, nt, xst in xtiles:
                    sl = slice(it0 * BH, (it0 + nt) * BH)
                    nc.scalar.activation(
                        xt[:, sl],
                        xst[:],
                        mybir.ActivationFunctionType.Tanh,
                        bias=tanh_bias,
                        scale=tanh_scale,
                    )
                    if tr1_eng == "vector":
                        nc.vector.tensor_copy(tr1[:, sl], xt[:, sl])
                    else:
                        nc.scalar.activation(
                            tr1[:, sl], xt[:, sl], mybir.ActivationFunctionType.Copy
                        )

            def alloc_xt(h):
                xt = xtp.tile([P, NI * BH], F32, tag="xt", name=f"xt_{h}")
                tr1 = trp.tile([P, NI * BH], BF16, tag="tr1", name=f"tr_{h}_1", bufs=2)
                return xt, tr1

            def emit_matmuls(accs, wr_slabs, d, tr_d):
                if d == DEG:
                    # whole last degree ot-major: each acc's accumulation
                    # closes early so psum evac + y DMA overlap the stream
                    for ot in range(NO):
                        for si, (it0, nt, wst) in enumerate(wr_slabs):
                            for il in range(nt):
                                it = it0 + il
                                nc.tensor.matmul(
                                    accs[ot][:],
                                    wst[:, il * OC + ot * P : il * OC + (ot + 1) * P],
                                    tr_d[:, it * BH : (it + 1) * BH],
                                    start=False,
                                    stop=(it == NI - 1),
                                )
                    return
                for si, (it0, nt, wst) in enumerate(wr_slabs):
                    for il in range(nt):
                        it = it0 + il
                        rhs = tr_d[:, it * BH : (it + 1) * BH]
                        for ot in range(NO):
                            nc.tensor.matmul(
                                accs[ot][:],
                                wst[:, il * OC + ot * P : il * OC + (ot + 1) * P],
                                rhs,
                                start=(d == 1 and it == 0),
                                stop=False,
                            )

            def emit_d0(accs):
                # d0 fold: acc[ot] += V.T @ ones  (adds y0[o] to every b col)
                for ot in range(NO):
                    nc.tensor.matmul(
                        accs[ot][:],
                        v_bf[:, ot * P : (ot + 1) * P],
                        ones512[:],
                        start=False,
                        stop=False,
                    )

            # ---- head: x chunks on the scalar DMA queue (parallel to the
            # W stream on sync); all dma_starts queued before the tanh chain
            # so the scalar engine FIFO never blocks a transfer start ----
            xt0, tr1_0 = alloc_xt(0)
            x0_tiles = emit_x_dmas(0, chunks=[1], eng=nc.sync)
            d1_head = emit_w_slabs(0, 1, _D1_SLABS[:2])
            x0_tiles += emit_x_dmas(0, chunks=[1], it0=1, eng=nc.sync)
            x0_tiles += emit_x_dmas(0, chunks=[2, 2, 2], it0=2, eng=nc.scalar)
            emit_tanh_chunks(0, xt0, tr1_0, x0_tiles)

            # PE warm-up: dummy matmuls on the ones tile, gated only on the
            # memset, bridging until the first real matmul (~12us) so the
            # HAM clock gate opens (1.2 -> 2.4 GHz) with no idle window in
            # between — the real stream then starts at full clock. Two
            # alternating psum banks so fill/drain pipeline back-to-back.
            wus = [psp.tile([P, BH], F32, tag="ps", name=f"wu{j}") for j in range(2)]
            for k in range(12):
                nc.tensor.matmul(
                    wus[k % 2][:],
                    ones512[:, 0:P],
                    ones512[:],
                    start=(k < 2),
                    stop=(k >= 10),
                )

            xts = [(xt0, tr1_0), None]

            for h in range(2):
                if h == 0:
                    d1_slabs_h0 = d1_head + emit_w_slabs(
                        0, 1, _D1_SLABS[2:], it0=2, tag="1b"
                    )
                xt, tr1 = xts[h]
                # ---- accumulation psum tiles: y.T chunk per o-tile ----
                accs = [
                    psp.tile([P, BH], F32, tag="ps", name=f"acc_h{h}_o{ot}")
                    for ot in range(NO)
                ]

                # ---- degree loop (d0 folded mid-stream via emit_d0) ----
                t_m1 = xt  # T_{d-1} (fp32 slab)
                t_m2 = None  # T_{d-2}
                QS = NI * BH // 4
                for d in range(1, DEG + 1):
                    if d == 1:
                        tr_d = tr1
                    else:
                        tr_d = trp.tile(
                            [P, NI * BH], BF16, tag="tr", name=f"tr_{h}_{d}"
                        )
                        t_new = statep.tile(
                            [P, NI * BH], F32, tag="state", name=f"st_{h}_{d}"
                        )
                        for q in range(4):
                            sl = slice(q * QS, (q + 1) * QS)
                            prod = prodp.tile(
                                [P, QS], F32, tag="prod", name=f"prod_{h}_{d}_{q}"
                            )
                            nc.vector.scalar_tensor_tensor(
                                prod[:],
                                t_m1[:, sl],
                                2.0,
                                xt[:, sl],
                                AluOpType.mult,
                                AluOpType.mult,
                            )
                            if d == 2:
                                # T2 = 2*xt^2 - 1
                                nc.vector.tensor_scalar_sub(t_new[:, sl], prod[:], 1.0)
                            else:
                                nc.vector.tensor_sub(t_new[:, sl], prod[:], t_m2[:, sl])
                            if h == 1 and d == 2 and q < 2:
                                # boundary: scalar queue is draining h0's
                                # evacs; cast on DVE so d2's matmuls start
                                nc.vector.tensor_copy(tr_d[:, sl], t_new[:, sl])
                            else:
                                nc.scalar.activation(
                                    tr_d[:, sl],
                                    t_new[:, sl],
                                    mybir.ActivationFunctionType.Copy,
                                )
                        t_m2, t_m1 = t_m1, t_new

                    # ---- W stream + matmuls for this degree ----
                    if h == 0 and d == 1:
                        wr_slabs = d1_slabs_h0
                    elif h == 1 and d == 1:
                        wr_slabs = h1_d1_head + emit_w_slabs(
                            1, 1, _D1_SLABS[2:], it0=2, tag="1b"
                        )
                    else:
                        wr_slabs = emit_w_slabs(h, d, _D1_SLABS if d == 1 else _D_SLABS)
                    emit_matmuls(accs, wr_slabs, d, tr_d)

                    if h == 0 and d == 2:
                        xt1, tr1_1 = alloc_xt(1)
                        x1_tiles = emit_x_dmas(1, chunks=[2, 2, 2, 2], eng=nc.sync)
                        emit_tanh_chunks(1, xt1, tr1_1, x1_tiles, tr1_eng="scalar")
                        xts[1] = (xt1, tr1_1)
                    if h == 0 and d == 3:
                        # W0 stream for the V-fold, after d3's slabs (the DMA
                        # queue has plenty of slack mid-stream)
                        w0_slabs = emit_w_slabs(0, 0, _D_SLABS, tag="w0")
                    if h == 0 and d == 4:
                        # V = sum of W0's 8 i-tiles (DVE), then cast to bf16
                        vtmp = [
                            prodp.tile([P, OC], F32, tag="vtmp", name=f"v_{j}", bufs=2)
                            for j in range(2)
                        ]
                        w0_chunks = [
                            wst[:, il * OC : (il + 1) * OC]
                            for it0, nt, wst in w0_slabs
                            for il in range(nt)
                        ]
                        nc.vector.tensor_add(vtmp[0][:], w0_chunks[0], w0_chunks[1])
                        sel = 0
                        for k, ch in enumerate(w0_chunks[2:]):
                            sel = (k + 1) % 2
                            nc.vector.tensor_add(vtmp[sel][:], vtmp[k % 2][:], ch)
                        nc.vector.tensor_copy(v_bf[:], vtmp[sel][:])
                    if (h == 0 and d == 5) or (h == 1 and d == 2):
                        emit_d0(accs)
                    if h == 0 and d == 7:
                        # prefetch h1's first-degree head slabs so the
                        # boundary isn't gated on their DMA
                        h1_d1_head = emit_w_slabs(1, 1, _D1_SLABS[:2])

                # ---- evacuate psum -> SBUF -> y.T ----
                # All evacs on scalar: the DVE FIFO is recurrence-critical at
                # the half boundary and queued evac copies there stall h1's
                # psum banks. h0: y DMAs on the scalar queue (sync must keep
                # feeding h1's W stream). h1 (tail): split across both queues.
                for ot in range(NO):
                    ev = evacp.tile([P, BH], F32, tag="evac", name=f"ev_h{h}_o{ot}")
                    nc.scalar.activation(
                        ev[:], accs[ot][:], mybir.ActivationFunctionType.Copy
                    )
                    if h == 1 and ot >= NO - 2:
                        # tail: split the final transfers across both queues
                        nc.sync.dma_start(
                            yt_d[ot * P : (ot + 1) * P, BH : BH + BH // 2],
                            ev[:, : BH // 2],
                        )
                        nc.scalar.dma_start(
                            yt_d[ot * P : (ot + 1) * P, BH + BH // 2 :],
                            ev[:, BH // 2 :],
                        )
                        continue
                    dma_eng = nc.sync if (h == 1 and ot % 2 == 0) else nc.scalar
                    dma_eng.dma_start(
                        yt_d[ot * P : (ot + 1) * P, h * BH : (h + 1) * BH],
                        ev[:],
                    )

    nc.compile()
    return nc


_CACHE: dict = {}


def make_in_maps(x, w):
    w_bf = np.ascontiguousarray(np.asarray(w, dtype=np.float32)).astype(
        ml_dtypes.bfloat16
    )
    return [
        {"xT": np.ascontiguousarray(x[c * BC : (c + 1) * BC].T), "w": w_bf}
        for c in range(N_CORES)
    ]


def kernel(x, cheby_coeffs, tanh_scale, tanh_bias):
    x = np.ascontiguousarray(np.asarray(x, dtype=np.float32))
    ts = float(np.asarray(tanh_scale))
    tb = float(np.asarray(tanh_bias))

    key = (ts, tb)
    if key not in _CACHE:
        _CACHE[key] = _build(ts, tb)
    nc = _CACHE[key]

    in_maps = make_in_maps(x, cheby_coeffs)
    res = bass_utils.run_bass_kernel_spmd(
        nc, in_maps, core_ids=list(range(N_CORES)), trace=False
    )

    y = np.empty((B, OC), dtype=np.float32)
    for c in range(N_CORES):
        y[c * BC : (c + 1) * BC, :] = res.results[c]["yt"].T
    return y


# revision 32
# speedup vs baseline: 1.0524x; 1.0088x over previous
"""ChebyKAN layer on 8 Trainium2 NeuronCores.

y = einsum('dbi,dio->bo', cheby_basis(tanh(x)), cheby_coeffs)

Strategy (per core, data-parallel over batch):
  - each core takes 1024 rows of x (8192/8) and the full coeffs
  - x arrives pre-transposed ([i, b] layout) and bf16-cast from the
    host (rel err 4.4e-3 simulated end-to-end, well under the 2e-2
    gate); tanh on the scalar engine computes in fp32
  - cheby_coeffs arrive as bf16 (host cast): halves the dominant W DMA
    stream (the W tensor is streamed once per batch-half) and the bf16
    stationary operand gets fast weight load on the PE
  - Chebyshev basis built on-the-fly in fp32 on the vector engine and
    cast to bf16 (the BIR verifier requires both matmul operands to
    share a dtype; bf16 runs full rate, rel err ~2e-3 vs the 2e-2 gate)
  - degree 0 (T0 == 1) is folded via V-fold: V[i',o] = sum_k W0[i'+128k,o]
    is pre-reduced on the vector engine (7 adds, off the PE), and each
    half accumulates it with 8 mid-stream start=False matmuls
    V.T @ ones[128,512]; replaces 128 full matmuls (1/9 of PE work)
    with 16 and adds zero head-of-pipe serialization
  - contraction: stationary = W[d, i-tile, o-tile] bf16, moving =
    T_d[i-tile, b-half] bf16; psum holds y.T chunks
    [o-tile 128, b-half 512] x 8 o-tiles = 8 banks
  - two b-halves of 512; W streamed from HBM once per half (bf16)
  - the last degree of each half runs ot-major so psum banks close
    progressively: evacuation + output DMA overlap the matmul stream
  - output is y.T per core; host transposes and concatenates
"""

import numpy as np
import ml_dtypes

import concourse.bass as bass
import concourse.tile as tile
from concourse import bacc, mybir
from concourse import bass_utils
from concourse.alu_op_type import AluOpType

N_CORES = 8
B = 8192
IC = 1024
OC = 1024
DEG = 8  # polynomial degree; DEG+1 = 9 basis terms
BC = B // N_CORES  # 1024 batch rows per core
P = 128
NI = IC // P  # 8 i-tiles
NO = OC // P  # 8 o-tiles
BH = BC // 2  # 512, b-half
F32 = mybir.dt.float32
BF16 = mybir.dt.bfloat16

# W slab granularity (in i-tiles): first degree of a half uses small
# head slabs so its first matmul's W-DMA chain is short.
_D1_SLABS = [1, 1, 2, 2, 2]
_D_SLABS = [2, 2, 2, 2]


def _build(tanh_scale: float, tanh_bias: float):
    nc = bacc.Bacc("TRN2", target_bir_lowering=False, debug=False, num_devices=N_CORES)

    xT_d = nc.dram_tensor("xT", [IC, BC], BF16, kind="ExternalInput").ap()
    w_d = nc.dram_tensor("w", [DEG + 1, IC, OC], BF16, kind="ExternalInput").ap()
    yt_d = nc.dram_tensor("yt", [OC, BC], F32, kind="ExternalOutput").ap()

    with tile.TileContext(nc) as tc:
        with (
            tc.tile_pool(name="const", bufs=1) as constp,
            tc.tile_pool(name="xin", bufs=5) as xinp,
            tc.tile_pool(name="xt", bufs=2) as xtp,
            tc.tile_pool(name="state", bufs=3) as statep,
            tc.tile_pool(name="prod", bufs=2) as prodp,
            tc.tile_pool(name="tr", bufs=3) as trp,
            tc.tile_pool(name="wstage", bufs=8) as wstagep,
            tc.tile_pool(name="evac", bufs=3) as evacp,
            tc.tile_pool(name="ps", bufs=8, space=bass.MemorySpace.PSUM) as psp,
        ):
            ones512 = constp.tile([P, BH], BF16, tag="ones")
            nc.vector.memset(ones512[:], 1.0)
            v_bf = constp.tile([P, OC], BF16, tag="vbf")  # V: d0 fold, bf16

            def emit_w_slabs(h, d, slab_sizes, it0=0, tag=None):
                """DMA W[d] i-tile slabs (bf16, used directly as stationary);
                returns [(first_it, ntiles, tile), ...]."""
                out = []
                for ws, nt in enumerate(slab_sizes):
                    wst = wstagep.tile(
                        [P, nt * OC],
                        BF16,
                        tag="wstage",
                        name=f"wst_{h}_{tag or d}_{ws}_{it0}",
                    )
                    nc.sync.dma_start(
                        wst[:].rearrange("p (il o) -> p il o", il=nt),
                        w_d[d, it0 * P : (it0 + nt) * P, :].rearrange(
                            "(il p) o -> p il o", p=P
                        ),
                    )
                    out.append((it0, nt, wst))
                    it0 += nt
                return out

            # ---- x.T chunk DMAs (batched up-front on a chosen queue) ----
            def emit_x_dmas(h, chunks, it0=0, eng=None):
                eng = eng or nc.sync
                tiles = []
                for nt in chunks:
                    xst = xinp.tile(
                        [P, nt * BH], BF16, tag="xin", name=f"xs_{h}_{it0}"
                    )
                    eng.dma_start(
                        xst[:].rearrange("p (il b) -> p il b", il=nt),
                        xT_d[
                            it0 * P : (it0 + nt) * P, h * BH : (h + 1) * BH
                        ].rearrange("(il p) b -> p il b", p=P),
                    )
                    tiles.append((it0, nt, xst))
                    it0 += nt
                return tiles

            # ---- tanh (scalar) + T1 bf16 cast per staged chunk ----
            def emit_tanh_chunks(h, xt, tr1, xtiles, tr1_eng="vector"):
                for it0, nt, xst in xtiles:
                    sl = slice(it0 * BH, (it0 + nt) * BH)
                    nc.scalar.activation(
                        xt[:, sl],
                        xst[:],
                        mybir.ActivationFunctionType.Tanh,
                        bias=tanh_bias,
                        scale=tanh_scale,
                    )
                    if tr1_eng == "vector":
                        nc.vector.tensor_copy(tr1[:, sl], xt[:, sl])
                    else:
                        nc.scalar.activation(
                            tr1[:, sl], xt[:, sl], mybir.ActivationFunctionType.Copy
                        )

            def alloc_xt(h):
                xt = xtp.tile([P, NI * BH], F32, tag="xt", name=f"xt_{h}")
                tr1 = trp.tile([P, NI * BH], BF16, tag="tr1", name=f"tr_{h}_1", bufs=2)
                return xt, tr1

            def emit_matmuls(accs, wr_slabs, d, tr_d):
                if d == DEG:
                    # whole last degree ot-major: each acc's accumulation
                    # closes early so psum evac + y DMA overlap the stream
                    for ot in range(NO):
                        for si, (it0, nt, wst) in enumerate(wr_slabs):
                            for il in range(nt):
                                it = it0 + il
                                nc.tensor.matmul(
                                    accs[ot][:],
                                    wst[:, il * OC + ot * P : il * OC + (ot + 1) * P],
                                    tr_d[:, it * BH : (it + 1) * BH],
                                    start=False,
                                    stop=(it == NI - 1),
                                )
                    return
                for si, (it0, nt, wst) in enumerate(wr_slabs):
                    for il in range(nt):
                        it = it0 + il
                        rhs = tr_d[:, it * BH : (it + 1) * BH]
                        for ot in range(NO):
                            nc.tensor.matmul(
                                accs[ot][:],
                                wst[:, il * OC + ot * P : il * OC + (ot + 1) * P],
                                rhs,
                                start=(d == 1 and it == 0),
                                stop=False,
                            )

            def emit_d0(accs):
                # d0 fold: acc[ot] += V.T @ ones  (adds y0[o] to every b col)
                for ot in range(NO):
                    nc.tensor.matmul(
                        accs[ot][:],
                        v_bf[:, ot * P : (ot + 1) * P],
                        ones512[:],
                        start=False,
                        stop=False,
                    )

            # ---- head: x chunks on the scalar DMA queue (parallel to the
            # W stream on sync); all dma_starts queued before the tanh chain
            # so the scalar engine FIFO never blocks a transfer start ----
            xt0, tr1_0 = alloc_xt(0)
            x0_tiles = emit_x_dmas(0, chunks=[1], eng=nc.sync)
            d1_head = emit_w_slabs(0, 1, _D1_SLABS[:2])
            x0_tiles += emit_x_dmas(0, chunks=[1], it0=1, eng=nc.sync)
            x0_tiles += emit_x_dmas(0, chunks=[2, 2, 2], it0=2, eng=nc.scalar)
            emit_tanh_chunks(0, xt0, tr1_0, x0_tiles)

            # PE warm-up: dummy matmuls on the ones tile, gated only on the
            # memset, bridging until the first real matmul (~12us) so the
            # HAM clock gate opens (1.2 -> 2.4 GHz) with no idle window in
            # between — the real stream then starts at full clock. Two
            # alternating psum banks so fill/drain pipeline back-to-back.
            wus = [psp.tile([P, BH], F32, tag="ps", name=f"wu{j}") for j in range(2)]
            for k in range(12):
                nc.tensor.matmul(
                    wus[k % 2][:],
                    ones512[:, 0:P],
                    ones512[:],
                    start=(k < 2),
                    stop=(k >= 10),
                )

            xts = [(xt0, tr1_0), None]

            for h in range(2):
                if h == 0:
                    d1_slabs_h0 = d1_head + emit_w_slabs(
                        0, 1, _D1_SLABS[2:], it0=2, tag="1b"
                    )
                xt, tr1 = xts[h]
                # ---- accumulation psum tiles: y.T chunk per o-tile ----
                accs = [
                    psp.tile([P, BH], F32, tag="ps", name=f"acc_h{h}_o{ot}")
                    for ot in range(NO)
                ]

                # ---- degree loop (d0 folded mid-stream via emit_d0) ----
                t_m1 = xt  # T_{d-1} (fp32 slab)
                t_m2 = None  # T_{d-2}
                QS = NI * BH // 4
                for d in range(1, DEG + 1):
                    if d == 1:
                        tr_d = tr1
                    else:
                        tr_d = trp.tile(
                            [P, NI * BH], BF16, tag="tr", name=f"tr_{h}_{d}"
                        )
                        t_new = statep.tile(
                            [P, NI * BH], F32, tag="state", name=f"st_{h}_{d}"
                        )
                        for q in range(4):
                            sl = slice(q * QS, (q + 1) * QS)
                            prod = prodp.tile(
                                [P, QS], F32, tag="prod", name=f"prod_{h}_{d}_{q}"
                            )
                            nc.vector.scalar_tensor_tensor(
                                prod[:],
                                t_m1[:, sl],
                                2.0,
                                xt[:, sl],
                                AluOpType.mult,
                                AluOpType.mult,
                            )
                            if d == 2:
                                # T2 = 2*xt^2 - 1
                                nc.vector.tensor_scalar_sub(t_new[:, sl], prod[:], 1.0)
                            else:
                                nc.vector.tensor_sub(t_new[:, sl], prod[:], t_m2[:, sl])
                            if h == 1 and d == 2 and q < 2:
                                # boundary: scalar queue is draining h0's
                                # evacs; cast on DVE so d2's matmuls start
                                nc.vector.tensor_copy(tr_d[:, sl], t_new[:, sl])
                            else:
                                nc.scalar.activation(
                                    tr_d[:, sl],
                                    t_new[:, sl],
                                    mybir.ActivationFunctionType.Copy,
                                )
                        t_m2, t_m1 = t_m1, t_new

                    # ---- W stream + matmuls for this degree ----
                    if h == 0 and d == 1:
                        wr_slabs = d1_slabs_h0
                    elif h == 1 and d == 1:
                        wr_slabs = h1_d1_head + emit_w_slabs(
                            1, 1, _D1_SLABS[2:], it0=2, tag="1b"
                        )
                    else:
                        wr_slabs = emit_w_slabs(h, d, _D1_SLABS if d == 1 else _D_SLABS)
                    emit_matmuls(accs, wr_slabs, d, tr_d)

                    if h == 0 and d == 2:
                        xt1, tr1_1 = alloc_xt(1)
                        x1_tiles = emit_x_dmas(1, chunks=[2, 2, 2, 2], eng=nc.sync)
                        emit_tanh_chunks(1, xt1, tr1_1, x1_tiles, tr1_eng="scalar")
                        xts[1] = (xt1, tr1_1)
                    if h == 0 and d == 3:
                        # W0 stream for the V-fold, after d3's slabs (the DMA
                        # queue has plenty of slack mid-stream)
                        w0_slabs = emit_w_slabs(0, 0, _D_SLABS, tag="w0")
                    if h == 0 and d == 4:
                        # V = sum of W0's 8 i-tiles (DVE), then cast to bf16
                        vtmp = [
                            prodp.tile([P, OC], F32, tag="vtmp", name=f"v_{j}", bufs=2)
                            for j in range(2)
                        ]
                        w0_chunks = [
                            wst[:, il * OC : (il + 1) * OC]
                            for it0, nt, wst in w0_slabs
                            for il in range(nt)
                        ]
                        nc.vector.tensor_add(vtmp[0][:], w0_chunks[0], w0_chunks[1])
                        sel = 0
                        for k, ch in enumerate(w0_chunks[2:]):
                            sel = (k + 1) % 2
                            nc.vector.tensor_add(vtmp[sel][:], vtmp[k % 2][:], ch)
                        nc.vector.tensor_copy(v_bf[:], vtmp[sel][:])
                    if (h == 0 and d == 5) or (h == 1 and d == 2):
                        emit_d0(accs)
                    if h == 0 and d == 7:
                        # prefetch h1's first-degree head slabs so the
                        # boundary isn't gated on their DMA
                        h1_d1_head = emit_w_slabs(1, 1, _D1_SLABS[:2])

                # ---- evacuate psum -> SBUF -> y.T ----
                # All evacs on scalar: the DVE FIFO is recurrence-critical at
                # the half boundary and queued evac copies there stall h1's
                # psum banks. h0: y DMAs on the scalar queue (sync must keep
                # feeding h1's W stream). h1 (tail): split across both queues.
                for ot in range(NO):
                    ev = evacp.tile([P, BH], F32, tag="evac", name=f"ev_h{h}_o{ot}")
                    nc.scalar.activation(
                        ev[:], accs[ot][:], mybir.ActivationFunctionType.Copy
                    )
                    if h == 1 and ot >= NO - 2:
                        # tail: split the final transfers across both queues
                        nc.sync.dma_start(
                            yt_d[ot * P : (ot + 1) * P, BH : BH + BH // 2],
                            ev[:, : BH // 2],
                        )
                        nc.scalar.dma_start(
                            yt_d[ot * P : (ot + 1) * P, BH + BH // 2 :],
                            ev[:, BH // 2 :],
                        )
                        continue
                    dma_eng = nc.sync if (h == 1 and ot % 2 == 0) else nc.scalar
                    dma_eng.dma_start(
                        yt_d[ot * P : (ot + 1) * P, h * BH : (h + 1) * BH],
                        ev[:],
                    )

    nc.compile()
    return nc


_CACHE: dict = {}


def make_in_maps(x, w):
    w_bf = np.ascontiguousarray(np.asarray(w, dtype=np.float32)).astype(
        ml_dtypes.bfloat16
    )
    x_bf = np.asarray(x, dtype=np.float32).astype(ml_dtypes.bfloat16)
    return [
        {"xT": np.ascontiguousarray(x_bf[c * BC : (c + 1) * BC].T), "w": w_bf}
        for c in range(N_CORES)
    ]


def kernel(x, cheby_coeffs, tanh_scale, tanh_bias):
    x = np.ascontiguousarray(np.asarray(x, dtype=np.float32))
    ts = float(np.asarray(tanh_scale))
    tb = float(np.asarray(tanh_bias))

    key = (ts, tb)
    if key not in _CACHE:
        _CACHE[key] = _build(ts, tb)
    nc = _CACHE[key]

    in_maps = make_in_maps(x, cheby_coeffs)
    res = bass_utils.run_bass_kernel_spmd(
        nc, in_maps, core_ids=list(range(N_CORES)), trace=False
    )

    y = np.empty((B, OC), dtype=np.float32)
    for c in range(N_CORES):
        y[c * BC : (c + 1) * BC, :] = res.results[c]["yt"].T
    return y
